# revision 11
# baseline (speedup 1.0000x reference)
"""Trainium2 Bass kernel for BinarizedInputNetwork.

Contract: kernel(**inputs) takes the FULL unsharded inputs (batch 128) and
returns the FULL [128, 12] float32 softmax output. Internally shards the
batch across 8 NeuronCores (16 images each), runs one SPMD Bass program.

Network (per image, input [1,128,128]):
  conv1 3x3 s2 p1 (1->64)  + BN + ReLU -> sign       => binary acts {0,1}
  conv2 3x3 s1 p1 (64->128, sign wts)  + BN + ReLU -> sign
  conv3 3x3 s2 p1 (128->128, sign wts) + BN + ReLU -> sign
  conv4 3x3 s1 p1 (128->192, sign wts) + BN + ReLU -> sign
  conv5 1x1 s1 p0 (192->192, sign wts) + BN + ReLU
  conv6 1x1 (192->12) + b ; GAP ; FC 12x12 + b ; softmax

Device mapping:
  - Convs are shifted matmuls: activations live in SBUF as [C, Hp*Wp]
    (channel on partition, zero-padded spatial), each 3x3 tap is one
    accumulating matmul (K=Cin, M=Cout, N=512 output positions).
  - sign(relu(bn(x))) == (x > t_c) for per-channel threshold t_c (bn scale
    is positive), done as a single VectorE tensor_scalar is_gt PSUM->SBUF
    (bf16 {0,1} acts; sign weights are exactly representable -> layers 2-5
    are numerically exact integer arithmetic).
  - L2 packs 2 taps per matmul (K=2*64): second copy of the input stored at
    partitions 64..127 shifted down one row.
  - conv6+GAP+FC folded: GAP sums via ScalarE activation accum_out, then
    logits = (fc_w@conv6_w/1024) @ sums + (fc_w@conv6_b + fc_b) as one tiny
    matmul with an extra ones-row; softmax on device.
"""

import sys

sys.path.insert(0, "/opt/trn_rl_repo")

import numpy as np

import concourse.bass as bass
import concourse.mybir as mybir
import concourse.bacc as bacc
import concourse.tile as tile
from concourse.bass_utils import run_bass_kernel_spmd

F32 = mybir.dt.float32
BF16 = mybir.dt.bfloat16
AX = mybir.AxisListType
OP = mybir.AluOpType
ACT = mybir.ActivationFunctionType

N_CORES = 8
B = 16  # images per core

EPS = 1e-5

# geometry
H1, W1 = 64, 64          # conv1 output spatial
P1 = W1 + 2              # padded width/height for A1/A2 (66)
S1 = P1 * P1             # 4356
H3, W3 = 32, 32          # conv3 output spatial
P3 = W3 + 2              # 34
S3 = P3 * P3             # 1156
NPOS = H3 * W3           # 1024 valid positions for L5/GAP

_CACHE = {}


def _build():
    """Trace + compile the Bass program (cached)."""
    if "nc" in _CACHE:
        return _CACHE

    nc = bacc.Bacc("TRN2", target_bir_lowering=False, debug=False,
                   num_devices=N_CORES)

    # ---- DRAM I/O ----
    # host-side im2col of the 1-channel input: [B, 9, 64*64]
    dX = nc.dram_tensor("x", [B, 9, 4096], F32, kind="ExternalInput").ap()
    dW1T = nc.dram_tensor("w1t", [9, 64], F32, kind="ExternalInput").ap()
    dW2P = nc.dram_tensor("w2p", [128, 384], BF16, kind="ExternalInput").ap()
    dW2S = nc.dram_tensor("w2s", [64, 384], BF16, kind="ExternalInput").ap()
    dW3 = nc.dram_tensor("w3", [128, 1152], BF16, kind="ExternalInput").ap()
    dW4A = nc.dram_tensor("w4a", [128, 1152], BF16, kind="ExternalInput").ap()
    dW4B = nc.dram_tensor("w4b", [128, 576], BF16, kind="ExternalInput").ap()
    dW5A = nc.dram_tensor("w5a", [128, 192], BF16, kind="ExternalInput").ap()
    dW5B = nc.dram_tensor("w5b", [64, 192], BF16, kind="ExternalInput").ap()
    dT1 = nc.dram_tensor("t1", [64, 1], F32, kind="ExternalInput").ap()
    dT2 = nc.dram_tensor("t2", [128, 1], F32, kind="ExternalInput").ap()
    dT3 = nc.dram_tensor("t3", [128, 1], F32, kind="ExternalInput").ap()
    dT4a = nc.dram_tensor("t4a", [128, 1], F32, kind="ExternalInput").ap()
    dT4b = nc.dram_tensor("t4b", [64, 1], F32, kind="ExternalInput").ap()
    dA5a = nc.dram_tensor("a5a", [128, 1], F32, kind="ExternalInput").ap()
    dA5b = nc.dram_tensor("a5b", [64, 1], F32, kind="ExternalInput").ap()
    dB5a = nc.dram_tensor("b5a", [128, 1], F32, kind="ExternalInput").ap()
    dB5b = nc.dram_tensor("b5b", [64, 1], F32, kind="ExternalInput").ap()
    dWTa = nc.dram_tensor("wta", [128, 12], F32, kind="ExternalInput").ap()
    dWTb = nc.dram_tensor("wtb", [65, 12], F32, kind="ExternalInput").ap()
    dY = nc.dram_tensor("y", [B, 12], F32, kind="ExternalOutput").ap()

    with tile.TileContext(nc) as tc:
        with tc.tile_pool(name="const", bufs=1) as cp, \
             tc.tile_pool(name="work", bufs=2) as wp, \
             tc.tile_pool(name="psum", bufs=3, space="PSUM") as pp:

            def ctile(name, shape, dtype):
                return cp.tile(shape, dtype, tag=name, name=name)

            # ---- persistent weight/param tiles ----
            cW1T = ctile("cW1T", [9, 64], F32)
            cW2P = ctile("cW2P", [128, 384], BF16)
            cW2S = ctile("cW2S", [128, 384], BF16)   # rows 64..127 used
            cW3 = ctile("cW3", [128, 1152], BF16)
            cW4A = ctile("cW4A", [128, 1152], BF16)
            cW4B = ctile("cW4B", [128, 576], BF16)
            cW5A = ctile("cW5A", [128, 192], BF16)
            cW5B = ctile("cW5B", [64, 192], BF16)
            cT1 = ctile("cT1", [64, 1], F32)
            cT2 = ctile("cT2", [128, 1], F32)
            cT3 = ctile("cT3", [128, 1], F32)
            cT4a = ctile("cT4a", [128, 1], F32)
            cT4b = ctile("cT4b", [64, 1], F32)
            cA5a = ctile("cA5a", [128, 1], F32)
            cA5b = ctile("cA5b", [64, 1], F32)
            cB5a = ctile("cB5a", [128, 1], F32)
            cB5b = ctile("cB5b", [64, 1], F32)
            cWTa = ctile("cWTa", [128, 12], F32)
            cWTb = ctile("cWTb", [65, 12], F32)

            nc.sync.dma_start(cW1T[:], dW1T[:])
            nc.sync.dma_start(cW2P[:], dW2P[:])
            nc.sync.dma_start(cW2S[64:128, :], dW2S[:])
            nc.sync.dma_start(cW3[:], dW3[:])
            nc.sync.dma_start(cW4A[:], dW4A[:])
            nc.sync.dma_start(cW4B[:], dW4B[:])
            nc.sync.dma_start(cW5A[:], dW5A[:])
            nc.sync.dma_start(cW5B[:], dW5B[:])
            nc.sync.dma_start(cT1[:], dT1[:])
            nc.sync.dma_start(cT2[:], dT2[:])
            nc.sync.dma_start(cT3[:], dT3[:])
            nc.sync.dma_start(cT4a[:], dT4a[:])
            nc.sync.dma_start(cT4b[:], dT4b[:])
            nc.sync.dma_start(cA5a[:], dA5a[:])
            nc.sync.dma_start(cA5b[:], dA5b[:])
            nc.sync.dma_start(cB5a[:], dB5a[:])
            nc.sync.dma_start(cB5b[:], dB5b[:])
            nc.sync.dma_start(cWTa[:], dWTa[:])
            nc.sync.dma_start(cWTb[:], dWTb[:])

            # ---- persistent activation buffers (double-buffered by parity) ----
            IC = [ctile(f"IC{p}", [9, 4096], F32) for p in range(2)]
            A1 = [ctile(f"A1_{p}", [128, S1], BF16) for p in range(2)]
            A2 = [ctile(f"A2_{p}", [128, S1], BF16) for p in range(2)]
            A3 = [ctile(f"A3_{p}", [128, S3], BF16) for p in range(2)]
            A4a = [ctile(f"A4a_{p}", [128, NPOS], BF16) for p in range(2)]
            A4b = [ctile(f"A4b_{p}", [64, NPOS], BF16) for p in range(2)]
            MACCa = ctile("MACCa", [128, 2 * B], F32)
            MACCb = ctile("MACCb", [64, 2 * B], F32)
            Msum = ctile("Msum", [128, B], F32)
            MsumB = ctile("MsumB", [65, B], F32)

            # zero padding once; interiors are rewritten every image
            for p in range(2):
                nc.gpsimd.memset(A1[p][:], 0.0)
                nc.gpsimd.memset(A2[p][:], 0.0)
                nc.gpsimd.memset(A3[p][:], 0.0)
            nc.vector.memset(MsumB[64:65, :], 1.0)

            def a1v(p):
                return A1[p].rearrange("p (a b) -> p a b", b=P1)

            def a2v(p):
                return A2[p].rearrange("p (a b) -> p a b", b=P1)

            def a3v(p):
                return A3[p].rearrange("p (a b) -> p a b", b=P3)

            # ---------------- conv1 + binarize -> A1 (both copies) -------------
            def conv1_block(i):
                p = i % 2
                nc.sync.dma_start(IC[p][:], dX[i])
                # 8 chunks of 8 output rows
                for r in range(8):
                    ps = pp.tile([64, 512], F32, tag="mm", name=f"ps_c1_{i}_{r}")
                    nc.tensor.matmul(
                        ps[:], cW1T[:], IC[p][0:9, r * 512:(r + 1) * 512],
                        start=True, stop=True)
                    nc.vector.tensor_scalar(
                        a1v(p)[0:64, r * 8 + 1: r * 8 + 9, 1:65],
                        ps[:].rearrange("q (a b) -> q a b", b=64),
                        cT1[:], None, OP.is_gt)
                # copy2 at partitions 64..127, shifted down one row:
                # copy2[q] = copy1[q + P1]
                nc.sync.dma_start(A1[p][64:128, 0:S1 - P1], A1[p][0:64, P1:S1])

            # ---------------- layer bodies ------------------------------------
            def l2_block(i):
                p = i % 2
                for r in range(8):
                    y0 = r * 8
                    ps = pp.tile([128, 512], F32, tag="mm", name=f"ps_l2_{i}_{r}")
                    psv = ps[:].rearrange("q (a b) -> q a b", b=64)
                    for kx in range(3):   # pairs: taps (ky=0,kx)+(ky=1,kx), K=128
                        nc.tensor.matmul(
                            psv,
                            cW2P[:, kx * 128:(kx + 1) * 128],
                            a1v(p)[:, y0:y0 + 8, kx:kx + 64],
                            start=(kx == 0), stop=False)
                    for kx in range(3):   # solo taps (ky=2,kx), K=64 on upper half
                        nc.tensor.matmul(
                            psv,
                            cW2S[64:128, kx * 128:(kx + 1) * 128],
                            a1v(p)[64:128, y0 + 1:y0 + 9, kx:kx + 64],
                            start=False, stop=(kx == 2))
                    nc.vector.tensor_scalar(
                        a2v(p)[:, y0 + 1:y0 + 9, 1:65], psv,
                        cT2[:], None, OP.is_gt)

            def l3_block(i):
                p = i % 2
                for r in range(2):
                    y0 = r * 16
                    ps = pp.tile([128, 512], F32, tag="mm", name=f"ps_l3_{i}_{r}")
                    psv = ps[:].rearrange("q (a b) -> q a b", b=32)
                    for t in range(9):
                        ky, kx = t // 3, t % 3
                        nc.tensor.matmul(
                            psv,
                            cW3[:, t * 128:(t + 1) * 128],
                            a2v(p)[:, 2 * y0 + ky: 2 * y0 + ky + 31: 2,
                                   kx: kx + 63: 2],
                            start=(t == 0), stop=(t == 8))
                    nc.vector.tensor_scalar(
                        a3v(p)[:, y0 + 1:y0 + 17, 1:33], psv,
                        cT3[:], None, OP.is_gt)

            def l4_block(i):
                p = i % 2
                for r in range(2):
                    y0 = r * 16
                    psa = pp.tile([128, 512], F32, tag="mb", bufs=2,
                                  name=f"ps_l4a_{i}_{r}")
                    psb = pp.tile([64, 512], F32, tag="mb2", bufs=2,
                                  name=f"ps_l4b_{i}_{r}")
                    psav = psa[:].rearrange("q (a b) -> q a b", b=32)
                    psbv = psb[:].rearrange("q (a b) -> q a b", b=32)
                    for t in range(9):
                        ky, kx = t // 3, t % 3
                        rhs = a3v(p)[:, y0 + ky: y0 + ky + 16, kx: kx + 32]
                        nc.tensor.matmul(
                            psav, cW4A[:, t * 128:(t + 1) * 128], rhs,
                            start=(t == 0), stop=(t == 8))
                        nc.tensor.matmul(
                            psbv, cW4B[:, t * 64:(t + 1) * 64], rhs,
                            start=(t == 0), stop=(t == 8))
                    nc.vector.tensor_scalar(
                        A4a[p][:, y0 * 32: y0 * 32 + 512], psa[:],
                        cT4a[:], None, OP.is_gt)
                    nc.vector.tensor_scalar(
                        A4b[p][:, y0 * 32: y0 * 32 + 512], psb[:],
                        cT4b[:], None, OP.is_gt)

            def l5_block(i):
                p = i % 2
                for c in range(2):
                    sl = slice(c * 512, (c + 1) * 512)
                    psa = pp.tile([128, 512], F32, tag="mb", bufs=2,
                                  name=f"ps_l5a_{i}_{c}")
                    psb = pp.tile([64, 512], F32, tag="mb2", bufs=2,
                                  name=f"ps_l5b_{i}_{c}")
                    nc.tensor.matmul(psa[:], cW5A[:, 0:128], A4a[p][:, sl],
                                     start=True, stop=False)
                    nc.tensor.matmul(psa[:], cW5B[:, 0:128], A4b[p][:, sl],
                                     start=False, stop=True)
                    nc.tensor.matmul(psb[:], cW5A[:, 128:192], A4a[p][:, sl],
                                     start=True, stop=False)
                    nc.tensor.matmul(psb[:], cW5B[:, 128:192], A4b[p][:, sl],
                                     start=False, stop=True)
                    # h5 = relu(a5*conv + b5); GAP partial sums via accum_out
                    scra = wp.tile([128, 512], F32, tag="scr_a", name=f"scra_{i}_{c}")
                    scrb = wp.tile([64, 512], F32, tag="scr_b", name=f"scrb_{i}_{c}")
                    nc.scalar.activation(
                        scra[:], psa[:], ACT.Relu, bias=cB5a[:], scale=cA5a[:],
                        accum_out=MACCa[:, 2 * i + c: 2 * i + c + 1])
                    nc.scalar.activation(
                        scrb[:], psb[:], ACT.Relu, bias=cB5b[:], scale=cA5b[:],
                        accum_out=MACCb[:, 2 * i + c: 2 * i + c + 1])

            # ---------------- main pipeline -----------------------------------
            conv1_block(0)
            for i in range(B):
                if i + 1 < B:
                    conv1_block(i + 1)
                l2_block(i)
                l3_block(i)
                l4_block(i)
                l5_block(i)

            # ---------------- GAP/FC/softmax tail -----------------------------
            nc.vector.tensor_reduce(
                Msum[:, 0:B], MACCa[:].rearrange("p (i c) -> p i c", c=2),
                axis=AX.X, op=OP.add)
            nc.vector.tensor_reduce(
                MsumB[0:64, 0:B], MACCb[:].rearrange("p (i c) -> p i c", c=2),
                axis=AX.X, op=OP.add)

            psf = pp.tile([16, 12], F32, tag="fc", bufs=1, name="ps_fc")
            nc.tensor.matmul(psf[:], Msum[:, 0:B], cWTa[:],
                             start=True, stop=False)
            nc.tensor.matmul(psf[:], MsumB[:, 0:B], cWTb[:],
                             start=False, stop=True)

            negmax = cp.tile([16, 1], F32, tag="negmax", name="negmax")
            esum = cp.tile([16, 1], F32, tag="esum", name="esum")
            rsum = cp.tile([16, 1], F32, tag="rsum", name="rsum")
            etile = cp.tile([16, 12], F32, tag="etile", name="etile")
            yout = cp.tile([16, 12], F32, tag="yout", name="yout")

            nc.vector.tensor_reduce(negmax[:], psf[:], axis=AX.X, op=OP.max,
                                    negate=True)
            nc.scalar.activation(etile[:], psf[:], ACT.Exp, bias=negmax[:],
                                 scale=1.0, accum_out=esum[:])
            nc.vector.reciprocal(rsum[:], esum[:])
            nc.vector.tensor_scalar(yout[:], etile[:], rsum[:], None, OP.mult)
            nc.sync.dma_start(dY[:], yout[:])

    nc.compile()
    _CACHE["nc"] = nc
    return _CACHE


def _host_prep(inputs):
    """Fold BN into thresholds/affines; sign-binarize weights; build per-core
    input maps."""
    f32 = np.float32
    bf16 = mybir.dt.np(BF16)

    x = np.asarray(inputs["x"], f32)

    def inv(l):
        return (np.asarray(inputs[f"bn{l}_g"], f32)
                / np.sqrt(np.asarray(inputs[f"bn{l}_v"], f32) + np.float32(EPS)))

    invs = {l: inv(l) for l in (1, 2, 3, 4, 5)}
    for l in (1, 2, 3, 4):
        assert (invs[l] > 0).all(), f"bn{l} scale not positive"

    def thr(l):
        return (np.asarray(inputs[f"bn{l}_m"], f32)
                - np.asarray(inputs[f"bn{l}_b"], f32) / invs[l])

    t1 = (thr(1) - np.asarray(inputs["conv1_b"], f32)).reshape(64, 1)
    t2 = thr(2).reshape(128, 1)
    t3 = thr(3).reshape(128, 1)
    t4 = thr(4)
    a5 = invs[5]
    b5 = (np.asarray(inputs["bn5_b"], f32)
          - np.asarray(inputs["bn5_m"], f32) * invs[5])

    # conv1 weights -> lhsT [tap, cout]
    w1 = np.asarray(inputs["conv1_w"], f32)           # [64,1,3,3]
    w1t = np.ascontiguousarray(
        w1[:, 0].reshape(64, 9).T)                     # [9, 64]

    sw2 = np.sign(np.asarray(inputs["w2"], f32))       # [128,64,3,3]
    sw3 = np.sign(np.asarray(inputs["w3"], f32))       # [128,128,3,3]
    sw4 = np.sign(np.asarray(inputs["w4"], f32))       # [192,128,3,3]
    sw5 = np.sign(np.asarray(inputs["w5"], f32))       # [192,192,1,1]

    # W2P[ci + 64*ky, kx*128 + co] for ky in {0,1}
    w2p = np.zeros((128, 384), f32)
    w2s = np.zeros((64, 384), f32)
    for kx in range(3):
        for ky in range(2):
            w2p[64 * ky:64 * (ky + 1), kx * 128:(kx + 1) * 128] = \
                sw2[:, :, ky, kx].T
        w2s[:, kx * 128:(kx + 1) * 128] = sw2[:, :, 2, kx].T

    w3t = np.zeros((128, 1152), f32)
    for t in range(9):
        w3t[:, t * 128:(t + 1) * 128] = sw3[:, :, t // 3, t % 3].T

    w4a = np.zeros((128, 1152), f32)
    w4b = np.zeros((128, 576), f32)
    for t in range(9):
        w4a[:, t * 128:(t + 1) * 128] = sw4[:128, :, t // 3, t % 3].T
        w4b[:, t * 64:(t + 1) * 64] = sw4[128:, :, t // 3, t % 3].T

    w5 = sw5[:, :, 0, 0]                               # [co=192, ci=192]
    w5a = np.ascontiguousarray(w5[:, :128].T)          # [128, 192]
    w5b = np.ascontiguousarray(w5[:, 128:].T)          # [64, 192]

    fc_w = np.asarray(inputs["fc_w"], f32)
    c6w = np.asarray(inputs["conv6_w"], f32)[:, :, 0, 0]   # [12, 192]
    Wp = (fc_w @ c6w) / np.float32(NPOS)               # [12, 192]
    cvec = fc_w @ np.asarray(inputs["conv6_b"], f32) + np.asarray(
        inputs["fc_b"], f32)                           # [12]
    wta = np.ascontiguousarray(Wp[:, :128].T)          # [128, 12]
    wtb = np.zeros((65, 12), f32)
    wtb[:64] = Wp[:, 128:].T
    wtb[64] = cvec

    shared = {
        "w1t": w1t.astype(f32),
        "w2p": w2p.astype(bf16), "w2s": w2s.astype(bf16),
        "w3": w3t.astype(bf16),
        "w4a": w4a.astype(bf16), "w4b": w4b.astype(bf16),
        "w5a": w5a.astype(bf16), "w5b": w5b.astype(bf16),
        "t1": t1.astype(f32), "t2": t2.astype(f32), "t3": t3.astype(f32),
        "t4a": t4[:128].reshape(128, 1).astype(f32),
        "t4b": t4[128:].reshape(64, 1).astype(f32),
        "a5a": a5[:128].reshape(128, 1).astype(f32),
        "a5b": a5[128:].reshape(64, 1).astype(f32),
        "b5a": b5[:128].reshape(128, 1).astype(f32),
        "b5b": b5[128:].reshape(64, 1).astype(f32),
        "wta": wta.astype(f32), "wtb": wtb.astype(f32),
    }
    # host im2col: cols[b, 3*ky+kx, y*64+x] = xpad[b, 2y+ky, 2x+kx]
    xpad = np.pad(x[:, 0], ((0, 0), (1, 1), (1, 1)))
    cols = np.stack([xpad[:, ky:ky + 127:2, kx:kx + 127:2]
                     for ky in range(3) for kx in range(3)],
                    axis=1).reshape(x.shape[0], 9, 4096)
    in_maps = []
    for c in range(N_CORES):
        m = dict(shared)
        m["x"] = np.ascontiguousarray(cols[c * B:(c + 1) * B])
        in_maps.append(m)
    return in_maps


def kernel(**inputs):
    cache = _build()
    in_maps = _host_prep(inputs)
    res = run_bass_kernel_spmd(cache["nc"], in_maps,
                               core_ids=list(range(N_CORES)))
    _CACHE["last_results"] = res
    return np.concatenate([res.results[c]["y"] for c in range(N_CORES)],
                          axis=0)


# ---------------------------------------------------------------------------
# numpy golden model of the device algorithm (for fast validation in test.py)
# ---------------------------------------------------------------------------
def golden(inputs):
    f32 = np.float32
    in_maps = _host_prep(inputs)
    outs = []
    for m in in_maps:
        cols = m["x"]  # [B, 9, 4096] host im2col
        t1 = m["t1"][:, 0]
        c1 = np.einsum("btn,tc->bcn", cols, m["w1t"]).reshape(-1, 64, 64, 64)
        a1 = (c1 > t1[None, :, None, None]).astype(f32)

        def bconv(a, wt, taps, stride, thr=None):
            # a: [B,C,H,W] binary; wt[ci, t*Cout:(t+1)*Cout]
            Bn, C, H, W = a.shape
            ap = np.pad(a, ((0, 0), (0, 0), (1, 1), (1, 1)))
            Ho, Wo = H // stride, W // stride
            Cout = wt.shape[1] // taps
            out = np.zeros((Bn, Cout, Ho, Wo), f32)
            for t in range(taps):
                ky, kx = t // 3, t % 3
                sl = ap[:, :, ky:ky + H:stride, kx:kx + W:stride][:, :, :Ho, :Wo]
                out += np.einsum("bcyx,cd->bdyx", sl,
                                 wt[:, t * Cout:(t + 1) * Cout].astype(f32))
            return out

        w2flat = np.zeros((64, 9 * 128), f32)
        for kx in range(3):
            for ky in range(2):
                t = 3 * ky + kx
                w2flat[:, t * 128:(t + 1) * 128] = \
                    m["w2p"][64 * ky:64 * ky + 64,
                             kx * 128:(kx + 1) * 128].astype(f32)
            t = 6 + kx
            w2flat[:, t * 128:(t + 1) * 128] = \
                m["w2s"][:, kx * 128:(kx + 1) * 128].astype(f32)
        c2 = bconv(a1, w2flat, 9, 1)
        a2 = (c2 > m["t2"].reshape(1, 128, 1, 1)).astype(f32)
        c3 = bconv(a2, m["w3"].astype(f32), 9, 2)
        a3 = (c3 > m["t3"].reshape(1, 128, 1, 1)).astype(f32)
        c4a = bconv(a3, m["w4a"].astype(f32), 9, 1)
        c4b = bconv(a3, m["w4b"].astype(f32), 9, 1)
        a4 = np.concatenate([
            (c4a > m["t4a"].reshape(1, 128, 1, 1)).astype(f32),
            (c4b > m["t4b"].reshape(1, 64, 1, 1)).astype(f32)], axis=1)
        w5 = np.concatenate([m["w5a"].astype(f32), m["w5b"].astype(f32)],
                            axis=0)  # [192, 192]
        c5 = np.einsum("bcyx,cd->bdyx", a4, w5)
        a5v = np.concatenate([m["a5a"], m["a5b"]], axis=0).reshape(1, 192, 1, 1)
        b5v = np.concatenate([m["b5a"], m["b5b"]], axis=0).reshape(1, 192, 1, 1)
        h5 = np.maximum(a5v * c5 + b5v, 0.0)
        sums = h5.sum(axis=(2, 3))  # [B, 192]
        WT = np.concatenate([m["wta"], m["wtb"][:64]], axis=0)  # [192, 12]
        logits = sums @ WT + m["wtb"][64][None, :]
        z = logits - logits.max(axis=1, keepdims=True)
        e = np.exp(z)
        outs.append(e / e.sum(axis=1, keepdims=True))
    return np.concatenate(outs, axis=0)


# revision 15
# speedup vs baseline: 1.5259x; 1.5259x over previous
"""Trainium2 Bass kernel for BinarizedInputNetwork.

Contract: kernel(**inputs) takes the FULL unsharded inputs (batch 128) and
returns the FULL [128, 12] float32 softmax output. Internally shards the
batch across 8 NeuronCores (16 images each), runs one SPMD Bass program.

Network (per image, input [1,128,128]):
  conv1 3x3 s2 p1 (1->64)  + BN + ReLU -> sign       => binary acts {0,1}
  conv2 3x3 s1 p1 (64->128, sign wts)  + BN + ReLU -> sign
  conv3 3x3 s2 p1 (128->128, sign wts) + BN + ReLU -> sign
  conv4 3x3 s1 p1 (128->192, sign wts) + BN + ReLU -> sign
  conv5 1x1 s1 p0 (192->192, sign wts) + BN + ReLU
  conv6 1x1 (192->12) + b ; GAP ; FC 12x12 + b ; softmax

Device mapping:
  - Convs are shifted matmuls: activations live in SBUF as [C, Hp*Wp]
    (channel on partition, zero-padded spatial), each 3x3 tap is one
    accumulating matmul (K=Cin, M=Cout, N=512 output positions).
  - sign(relu(bn(x))) == (x > t_c) for per-channel threshold t_c (bn scale
    is positive), done as a single VectorE tensor_scalar is_gt PSUM->SBUF
    (bf16 {0,1} acts; sign weights are exactly representable -> layers 2-5
    are numerically exact integer arithmetic).
  - L2 packs 2 taps per matmul (K=2*64): second copy of the input stored at
    partitions 64..127 shifted down one row.
  - conv6+GAP+FC folded: GAP sums via ScalarE activation accum_out, then
    logits = (fc_w@conv6_w/1024) @ sums + (fc_w@conv6_b + fc_b) as one tiny
    matmul with an extra ones-row; softmax on device.
"""

import sys

sys.path.insert(0, "/opt/trn_rl_repo")

import numpy as np

import concourse.bass as bass
import concourse.mybir as mybir
import concourse.bacc as bacc
import concourse.tile as tile
from concourse.bass_utils import run_bass_kernel_spmd

F32 = mybir.dt.float32
BF16 = mybir.dt.bfloat16
AX = mybir.AxisListType
OP = mybir.AluOpType
ACT = mybir.ActivationFunctionType

N_CORES = 8
B = 16  # images per core

EPS = 1e-5

# geometry
H1, W1 = 64, 64          # conv1 output spatial
P1 = W1 + 2              # padded width/height for A1/A2 (66)
S1 = P1 * P1             # 4356
H3, W3 = 32, 32          # conv3 output spatial
P3 = W3 + 2              # 34
S3 = P3 * P3             # 1156
NPOS = H3 * W3           # 1024 valid positions for L5/GAP

_CACHE = {}


def _build(reps=1):
    """Trace + compile the Bass program (cached). reps>1 replicates the whole
    pipeline on-device (for timing via wall-clock differencing)."""
    key = f"nc{reps}"
    if key in _CACHE:
        return _CACHE

    nc = bacc.Bacc("TRN2", target_bir_lowering=False, debug=False,
                   num_devices=N_CORES)

    # ---- DRAM I/O ----
    # host-side im2col of the 1-channel input: [B, 9, 64*64]
    dX = nc.dram_tensor("x", [B, 9, 4096], F32, kind="ExternalInput").ap()
    dW1T = nc.dram_tensor("w1t", [9, 64], F32, kind="ExternalInput").ap()
    dW2P = nc.dram_tensor("w2p", [128, 384], BF16, kind="ExternalInput").ap()
    dW2S = nc.dram_tensor("w2s", [64, 384], BF16, kind="ExternalInput").ap()
    dW3 = nc.dram_tensor("w3", [128, 1152], BF16, kind="ExternalInput").ap()
    dW4A = nc.dram_tensor("w4a", [128, 1152], BF16, kind="ExternalInput").ap()
    dW4B = nc.dram_tensor("w4b", [128, 576], BF16, kind="ExternalInput").ap()
    dW5A = nc.dram_tensor("w5a", [128, 192], BF16, kind="ExternalInput").ap()
    dW5B = nc.dram_tensor("w5b", [64, 192], BF16, kind="ExternalInput").ap()
    dT1 = nc.dram_tensor("t1", [64, 1], F32, kind="ExternalInput").ap()
    dT2 = nc.dram_tensor("t2", [128, 1], F32, kind="ExternalInput").ap()
    dT3 = nc.dram_tensor("t3", [128, 1], F32, kind="ExternalInput").ap()
    dT4a = nc.dram_tensor("t4a", [128, 1], F32, kind="ExternalInput").ap()
    dT4b = nc.dram_tensor("t4b", [64, 1], F32, kind="ExternalInput").ap()
    dA5a = nc.dram_tensor("a5a", [128, 1], F32, kind="ExternalInput").ap()
    dA5b = nc.dram_tensor("a5b", [64, 1], F32, kind="ExternalInput").ap()
    dB5a = nc.dram_tensor("b5a", [128, 1], F32, kind="ExternalInput").ap()
    dB5b = nc.dram_tensor("b5b", [64, 1], F32, kind="ExternalInput").ap()
    dWTa = nc.dram_tensor("wta", [128, 12], F32, kind="ExternalInput").ap()
    dWTb = nc.dram_tensor("wtb", [65, 12], F32, kind="ExternalInput").ap()
    dY = nc.dram_tensor("y", [B, 12], F32, kind="ExternalOutput").ap()

    with tile.TileContext(nc) as tc:
        with tc.tile_pool(name="const", bufs=1) as cp, \
             tc.tile_pool(name="work", bufs=2) as wp, \
             tc.tile_pool(name="psum", bufs=3, space="PSUM") as pp:

            def ctile(name, shape, dtype):
                return cp.tile(shape, dtype, tag=name, name=name)

            # ---- persistent weight/param tiles ----
            cW1T = ctile("cW1T", [9, 64], F32)
            cW2P = ctile("cW2P", [128, 384], BF16)
            cW2S = ctile("cW2S", [128, 384], BF16)   # rows 64..127 used
            cW3 = ctile("cW3", [128, 1152], BF16)
            cW4A = ctile("cW4A", [128, 1152], BF16)
            cW4B = ctile("cW4B", [128, 576], BF16)
            cW5A = ctile("cW5A", [128, 192], BF16)
            cW5B = ctile("cW5B", [64, 192], BF16)
            cT1 = ctile("cT1", [64, 1], F32)
            cT2 = ctile("cT2", [128, 1], F32)
            cT3 = ctile("cT3", [128, 1], F32)
            cT4a = ctile("cT4a", [128, 1], F32)
            cT4b = ctile("cT4b", [64, 1], F32)
            cA5a = ctile("cA5a", [128, 1], F32)
            cA5b = ctile("cA5b", [64, 1], F32)
            cB5a = ctile("cB5a", [128, 1], F32)
            cB5b = ctile("cB5b", [64, 1], F32)
            cWTa = ctile("cWTa", [128, 12], F32)
            cWTb = ctile("cWTb", [65, 12], F32)

            nc.sync.dma_start(cW1T[:], dW1T[:])
            nc.sync.dma_start(cW2P[:], dW2P[:])
            nc.sync.dma_start(cW2S[64:128, :], dW2S[:])
            nc.sync.dma_start(cW3[:], dW3[:])
            nc.sync.dma_start(cW4A[:], dW4A[:])
            nc.sync.dma_start(cW4B[:], dW4B[:])
            nc.sync.dma_start(cW5A[:], dW5A[:])
            nc.sync.dma_start(cW5B[:], dW5B[:])
            nc.sync.dma_start(cT1[:], dT1[:])
            nc.sync.dma_start(cT2[:], dT2[:])
            nc.sync.dma_start(cT3[:], dT3[:])
            nc.sync.dma_start(cT4a[:], dT4a[:])
            nc.sync.dma_start(cT4b[:], dT4b[:])
            nc.sync.dma_start(cA5a[:], dA5a[:])
            nc.sync.dma_start(cA5b[:], dA5b[:])
            nc.sync.dma_start(cB5a[:], dB5a[:])
            nc.sync.dma_start(cB5b[:], dB5b[:])
            nc.sync.dma_start(cWTa[:], dWTa[:])
            nc.sync.dma_start(cWTb[:], dWTb[:])

            # ---- persistent activation buffers (double-buffered by parity) ----
            IC = [ctile(f"IC{p}", [9, 4096], F32) for p in range(2)]
            A1 = [ctile(f"A1_{p}", [128, S1], BF16) for p in range(2)]
            A2 = [ctile(f"A2_{p}", [128, S1], BF16) for p in range(2)]
            A3 = [ctile(f"A3_{p}", [128, S3], BF16) for p in range(2)]
            A4a = [ctile(f"A4a_{p}", [128, NPOS], BF16) for p in range(2)]
            A4b = [ctile(f"A4b_{p}", [64, NPOS], BF16) for p in range(2)]
            MACCa = ctile("MACCa", [128, 2 * B], F32)
            MACCb = ctile("MACCb", [64, 2 * B], F32)
            Msum = ctile("Msum", [128, B], F32)
            MsumB = ctile("MsumB", [65, B], F32)

            # zero padding once; interiors are rewritten every image
            for p in range(2):
                nc.gpsimd.memset(A1[p][:], 0.0)
                nc.gpsimd.memset(A2[p][:], 0.0)
                nc.gpsimd.memset(A3[p][:], 0.0)
            nc.vector.memset(MsumB[64:65, :], 1.0)

            def a1v(p):
                return A1[p].rearrange("p (a b) -> p a b", b=P1)

            def a2v(p):
                return A2[p].rearrange("p (a b) -> p a b", b=P1)

            def a3v(p):
                return A3[p].rearrange("p (a b) -> p a b", b=P3)

            # ---------------- conv1 + binarize -> A1 (both copies) -------------
            def conv1_block(i):
                p = i % 2
                nc.sync.dma_start(IC[p][:], dX[i])
                # 8 chunks of 8 output rows
                for r in range(8):
                    ps = pp.tile([64, 512], F32, tag="mm", name=f"ps_c1_{i}_{r}")
                    nc.tensor.matmul(
                        ps[:], cW1T[:], IC[p][0:9, r * 512:(r + 1) * 512],
                        start=True, stop=True)
                    nc.vector.tensor_scalar(
                        a1v(p)[0:64, r * 8 + 1: r * 8 + 9, 1:65],
                        ps[:].rearrange("q (a b) -> q a b", b=64),
                        cT1[:], None, OP.is_gt)
                # copy2 at partitions 64..127, shifted down one row:
                # copy2[q] = copy1[q + P1]
                nc.sync.dma_start(A1[p][64:128, 0:S1 - P1], A1[p][0:64, P1:S1])

            # ---------------- layer bodies ------------------------------------
            def l2_block(i):
                p = i % 2
                for r in range(8):
                    y0 = r * 8
                    ps = pp.tile([128, 512], F32, tag="mm", name=f"ps_l2_{i}_{r}")
                    psv = ps[:].rearrange("q (a b) -> q a b", b=64)
                    for kx in range(3):   # pairs: taps (ky=0,kx)+(ky=1,kx), K=128
                        nc.tensor.matmul(
                            psv,
                            cW2P[:, kx * 128:(kx + 1) * 128],
                            a1v(p)[:, y0:y0 + 8, kx:kx + 64],
                            start=(kx == 0), stop=False)
                    for kx in range(3):   # solo taps (ky=2,kx), K=64 on upper half
                        nc.tensor.matmul(
                            psv,
                            cW2S[64:128, kx * 128:(kx + 1) * 128],
                            a1v(p)[64:128, y0 + 1:y0 + 9, kx:kx + 64],
                            start=False, stop=(kx == 2))
                    nc.vector.tensor_scalar(
                        a2v(p)[:, y0 + 1:y0 + 9, 1:65], psv,
                        cT2[:], None, OP.is_gt)

            def l3_block(i):
                p = i % 2
                for r in range(2):
                    y0 = r * 16
                    ps = pp.tile([128, 512], F32, tag="mm", name=f"ps_l3_{i}_{r}")
                    psv = ps[:].rearrange("q (a b) -> q a b", b=32)
                    for t in range(9):
                        ky, kx = t // 3, t % 3
                        nc.tensor.matmul(
                            psv,
                            cW3[:, t * 128:(t + 1) * 128],
                            a2v(p)[:, 2 * y0 + ky: 2 * y0 + ky + 31: 2,
                                   kx: kx + 63: 2],
                            start=(t == 0), stop=(t == 8))
                    nc.vector.tensor_scalar(
                        a3v(p)[:, y0 + 1:y0 + 17, 1:33], psv,
                        cT3[:], None, OP.is_gt)

            def l4_block(i):
                p = i % 2
                for r in range(2):
                    y0 = r * 16
                    psa = pp.tile([128, 512], F32, tag="mb", bufs=2,
                                  name=f"ps_l4a_{i}_{r}")
                    psb = pp.tile([64, 512], F32, tag="mb2", bufs=2,
                                  name=f"ps_l4b_{i}_{r}")
                    psav = psa[:].rearrange("q (a b) -> q a b", b=32)
                    psbv = psb[:].rearrange("q (a b) -> q a b", b=32)
                    for t in range(9):
                        ky, kx = t // 3, t % 3
                        rhs = a3v(p)[:, y0 + ky: y0 + ky + 16, kx: kx + 32]
                        nc.tensor.matmul(
                            psav, cW4A[:, t * 128:(t + 1) * 128], rhs,
                            start=(t == 0), stop=(t == 8))
                        nc.tensor.matmul(
                            psbv, cW4B[:, t * 64:(t + 1) * 64], rhs,
                            start=(t == 0), stop=(t == 8))
                    nc.vector.tensor_scalar(
                        A4a[p][:, y0 * 32: y0 * 32 + 512], psa[:],
                        cT4a[:], None, OP.is_gt)
                    nc.vector.tensor_scalar(
                        A4b[p][:, y0 * 32: y0 * 32 + 512], psb[:],
                        cT4b[:], None, OP.is_gt)

            def l5_block(i):
                p = i % 2
                for c in range(2):
                    sl = slice(c * 512, (c + 1) * 512)
                    psa = pp.tile([128, 512], F32, tag="mb", bufs=2,
                                  name=f"ps_l5a_{i}_{c}")
                    psb = pp.tile([64, 512], F32, tag="mb2", bufs=2,
                                  name=f"ps_l5b_{i}_{c}")
                    nc.tensor.matmul(psa[:], cW5A[:, 0:128], A4a[p][:, sl],
                                     start=True, stop=False)
                    nc.tensor.matmul(psa[:], cW5B[:, 0:128], A4b[p][:, sl],
                                     start=False, stop=True)
                    nc.tensor.matmul(psb[:], cW5A[:, 128:192], A4a[p][:, sl],
                                     start=True, stop=False)
                    nc.tensor.matmul(psb[:], cW5B[:, 128:192], A4b[p][:, sl],
                                     start=False, stop=True)
                    # h5 = relu(a5*conv + b5); GAP partial sums via accum_out
                    scra = wp.tile([128, 512], F32, tag="scr_a", name=f"scra_{i}_{c}")
                    scrb = wp.tile([64, 512], F32, tag="scr_b", name=f"scrb_{i}_{c}")
                    nc.scalar.activation(
                        scra[:], psa[:], ACT.Relu, bias=cB5a[:], scale=cA5a[:],
                        accum_out=MACCa[:, 2 * i + c: 2 * i + c + 1])
                    nc.scalar.activation(
                        scrb[:], psb[:], ACT.Relu, bias=cB5b[:], scale=cA5b[:],
                        accum_out=MACCb[:, 2 * i + c: 2 * i + c + 1])

            # ---------------- main pipeline -----------------------------------
            for _rep in range(reps):
                conv1_block(0)
                for i in range(B):
                    if i + 1 < B:
                        conv1_block(i + 1)
                    l2_block(i)
                    l3_block(i)
                    l4_block(i)
                    l5_block(i)

            # ---------------- GAP/FC/softmax tail -----------------------------
            nc.vector.tensor_reduce(
                Msum[:, 0:B], MACCa[:].rearrange("p (i c) -> p i c", c=2),
                axis=AX.X, op=OP.add)
            nc.vector.tensor_reduce(
                MsumB[0:64, 0:B], MACCb[:].rearrange("p (i c) -> p i c", c=2),
                axis=AX.X, op=OP.add)

            psf = pp.tile([16, 12], F32, tag="fc", bufs=1, name="ps_fc")
            nc.tensor.matmul(psf[:], Msum[:, 0:B], cWTa[:],
                             start=True, stop=False)
            nc.tensor.matmul(psf[:], MsumB[:, 0:B], cWTb[:],
                             start=False, stop=True)

            negmax = cp.tile([16, 1], F32, tag="negmax", name="negmax")
            esum = cp.tile([16, 1], F32, tag="esum", name="esum")
            rsum = cp.tile([16, 1], F32, tag="rsum", name="rsum")
            etile = cp.tile([16, 12], F32, tag="etile", name="etile")
            yout = cp.tile([16, 12], F32, tag="yout", name="yout")

            nc.vector.tensor_reduce(negmax[:], psf[:], axis=AX.X, op=OP.max,
                                    negate=True)
            nc.scalar.activation(etile[:], psf[:], ACT.Exp, bias=negmax[:],
                                 scale=1.0, accum_out=esum[:])
            nc.vector.reciprocal(rsum[:], esum[:])
            nc.vector.tensor_scalar(yout[:], etile[:], rsum[:], None, OP.mult)
            nc.sync.dma_start(dY[:], yout[:])

    nc.compile()
    _CACHE[key] = nc
    return _CACHE


def _host_prep(inputs):
    """Fold BN into thresholds/affines; sign-binarize weights; build per-core
    input maps."""
    f32 = np.float32
    bf16 = mybir.dt.np(BF16)

    x = np.asarray(inputs["x"], f32)

    def inv(l):
        return (np.asarray(inputs[f"bn{l}_g"], f32)
                / np.sqrt(np.asarray(inputs[f"bn{l}_v"], f32) + np.float32(EPS)))

    invs = {l: inv(l) for l in (1, 2, 3, 4, 5)}
    for l in (1, 2, 3, 4):
        assert (invs[l] > 0).all(), f"bn{l} scale not positive"

    def thr(l):
        return (np.asarray(inputs[f"bn{l}_m"], f32)
                - np.asarray(inputs[f"bn{l}_b"], f32) / invs[l])

    t1 = (thr(1) - np.asarray(inputs["conv1_b"], f32)).reshape(64, 1)
    t2 = thr(2).reshape(128, 1)
    t3 = thr(3).reshape(128, 1)
    t4 = thr(4)
    a5 = invs[5]
    b5 = (np.asarray(inputs["bn5_b"], f32)
          - np.asarray(inputs["bn5_m"], f32) * invs[5])

    # conv1 weights -> lhsT [tap, cout]
    w1 = np.asarray(inputs["conv1_w"], f32)           # [64,1,3,3]
    w1t = np.ascontiguousarray(
        w1[:, 0].reshape(64, 9).T)                     # [9, 64]

    sw2 = np.sign(np.asarray(inputs["w2"], f32))       # [128,64,3,3]
    sw3 = np.sign(np.asarray(inputs["w3"], f32))       # [128,128,3,3]
    sw4 = np.sign(np.asarray(inputs["w4"], f32))       # [192,128,3,3]
    sw5 = np.sign(np.asarray(inputs["w5"], f32))       # [192,192,1,1]

    # W2P[ci + 64*ky, kx*128 + co] for ky in {0,1}
    w2p = np.zeros((128, 384), f32)
    w2s = np.zeros((64, 384), f32)
    for kx in range(3):
        for ky in range(2):
            w2p[64 * ky:64 * (ky + 1), kx * 128:(kx + 1) * 128] = \
                sw2[:, :, ky, kx].T
        w2s[:, kx * 128:(kx + 1) * 128] = sw2[:, :, 2, kx].T

    w3t = np.zeros((128, 1152), f32)
    for t in range(9):
        w3t[:, t * 128:(t + 1) * 128] = sw3[:, :, t // 3, t % 3].T

    w4a = np.zeros((128, 1152), f32)
    w4b = np.zeros((128, 576), f32)
    for t in range(9):
        w4a[:, t * 128:(t + 1) * 128] = sw4[:128, :, t // 3, t % 3].T
        w4b[:, t * 64:(t + 1) * 64] = sw4[128:, :, t // 3, t % 3].T

    w5 = sw5[:, :, 0, 0]                               # [co=192, ci=192]
    w5a = np.ascontiguousarray(w5[:, :128].T)          # [128, 192]
    w5b = np.ascontiguousarray(w5[:, 128:].T)          # [64, 192]

    fc_w = np.asarray(inputs["fc_w"], f32)
    c6w = np.asarray(inputs["conv6_w"], f32)[:, :, 0, 0]   # [12, 192]
    Wp = (fc_w @ c6w) / np.float32(NPOS)               # [12, 192]
    cvec = fc_w @ np.asarray(inputs["conv6_b"], f32) + np.asarray(
        inputs["fc_b"], f32)                           # [12]
    wta = np.ascontiguousarray(Wp[:, :128].T)          # [128, 12]
    wtb = np.zeros((65, 12), f32)
    wtb[:64] = Wp[:, 128:].T
    wtb[64] = cvec

    shared = {
        "w1t": w1t.astype(f32),
        "w2p": w2p.astype(bf16), "w2s": w2s.astype(bf16),
        "w3": w3t.astype(bf16),
        "w4a": w4a.astype(bf16), "w4b": w4b.astype(bf16),
        "w5a": w5a.astype(bf16), "w5b": w5b.astype(bf16),
        "t1": t1.astype(f32), "t2": t2.astype(f32), "t3": t3.astype(f32),
        "t4a": t4[:128].reshape(128, 1).astype(f32),
        "t4b": t4[128:].reshape(64, 1).astype(f32),
        "a5a": a5[:128].reshape(128, 1).astype(f32),
        "a5b": a5[128:].reshape(64, 1).astype(f32),
        "b5a": b5[:128].reshape(128, 1).astype(f32),
        "b5b": b5[128:].reshape(64, 1).astype(f32),
        "wta": wta.astype(f32), "wtb": wtb.astype(f32),
    }
    # host im2col: cols[b, 3*ky+kx, y*64+x] = xpad[b, 2y+ky, 2x+kx]
    xpad = np.pad(x[:, 0], ((0, 0), (1, 1), (1, 1)))
    cols = np.stack([xpad[:, ky:ky + 127:2, kx:kx + 127:2]
                     for ky in range(3) for kx in range(3)],
                    axis=1).reshape(x.shape[0], 9, 4096)
    in_maps = []
    for c in range(N_CORES):
        m = dict(shared)
        m["x"] = np.ascontiguousarray(cols[c * B:(c + 1) * B])
        in_maps.append(m)
    return in_maps


def kernel(**inputs):
    cache = _build()
    in_maps = _host_prep(inputs)
    res = run_bass_kernel_spmd(cache["nc1"], in_maps,
                               core_ids=list(range(N_CORES)))
    _CACHE["last_results"] = res
    return np.concatenate([res.results[c]["y"] for c in range(N_CORES)],
                          axis=0)


# ---------------------------------------------------------------------------
# numpy golden model of the device algorithm (for fast validation in test.py)
# ---------------------------------------------------------------------------
def golden(inputs):
    f32 = np.float32
    in_maps = _host_prep(inputs)
    outs = []
    for m in in_maps:
        cols = m["x"]  # [B, 9, 4096] host im2col
        t1 = m["t1"][:, 0]
        c1 = np.einsum("btn,tc->bcn", cols, m["w1t"]).reshape(-1, 64, 64, 64)
        a1 = (c1 > t1[None, :, None, None]).astype(f32)

        def bconv(a, wt, taps, stride, thr=None):
            # a: [B,C,H,W] binary; wt[ci, t*Cout:(t+1)*Cout]
            Bn, C, H, W = a.shape
            ap = np.pad(a, ((0, 0), (0, 0), (1, 1), (1, 1)))
            Ho, Wo = H // stride, W // stride
            Cout = wt.shape[1] // taps
            out = np.zeros((Bn, Cout, Ho, Wo), f32)
            for t in range(taps):
                ky, kx = t // 3, t % 3
                sl = ap[:, :, ky:ky + H:stride, kx:kx + W:stride][:, :, :Ho, :Wo]
                out += np.einsum("bcyx,cd->bdyx", sl,
                                 wt[:, t * Cout:(t + 1) * Cout].astype(f32))
            return out

        w2flat = np.zeros((64, 9 * 128), f32)
        for kx in range(3):
            for ky in range(2):
                t = 3 * ky + kx
                w2flat[:, t * 128:(t + 1) * 128] = \
                    m["w2p"][64 * ky:64 * ky + 64,
                             kx * 128:(kx + 1) * 128].astype(f32)
            t = 6 + kx
            w2flat[:, t * 128:(t + 1) * 128] = \
                m["w2s"][:, kx * 128:(kx + 1) * 128].astype(f32)
        c2 = bconv(a1, w2flat, 9, 1)
        a2 = (c2 > m["t2"].reshape(1, 128, 1, 1)).astype(f32)
        c3 = bconv(a2, m["w3"].astype(f32), 9, 2)
        a3 = (c3 > m["t3"].reshape(1, 128, 1, 1)).astype(f32)
        c4a = bconv(a3, m["w4a"].astype(f32), 9, 1)
        c4b = bconv(a3, m["w4b"].astype(f32), 9, 1)
        a4 = np.concatenate([
            (c4a > m["t4a"].reshape(1, 128, 1, 1)).astype(f32),
            (c4b > m["t4b"].reshape(1, 64, 1, 1)).astype(f32)], axis=1)
        w5 = np.concatenate([m["w5a"].astype(f32), m["w5b"].astype(f32)],
                            axis=0)  # [192, 192]
        c5 = np.einsum("bcyx,cd->bdyx", a4, w5)
        a5v = np.concatenate([m["a5a"], m["a5b"]], axis=0).reshape(1, 192, 1, 1)
        b5v = np.concatenate([m["b5a"], m["b5b"]], axis=0).reshape(1, 192, 1, 1)
        h5 = np.maximum(a5v * c5 + b5v, 0.0)
        sums = h5.sum(axis=(2, 3))  # [B, 192]
        WT = np.concatenate([m["wta"], m["wtb"][:64]], axis=0)  # [192, 12]
        logits = sums @ WT + m["wtb"][64][None, :]
        z = logits - logits.max(axis=1, keepdims=True)
        e = np.exp(z)
        outs.append(e / e.sum(axis=1, keepdims=True))
    return np.concatenate(outs, axis=0)


# revision 26
# speedup vs baseline: 3.2715x; 2.1440x over previous
"""Trainium2 Bass kernel for BinarizedInputNetwork.

Contract: kernel(**inputs) takes the FULL unsharded inputs (batch 128) and
returns the FULL [128, 12] float32 softmax output. Internally shards the
batch across 8 NeuronCores (16 images each), runs one SPMD Bass program.

Network (per image, input [1,128,128]):
  conv1 3x3 s2 p1 (1->64)  + BN + ReLU -> sign       => binary acts {0,1}
  conv2 3x3 s1 p1 (64->128, sign wts)  + BN + ReLU -> sign
  conv3 3x3 s2 p1 (128->128, sign wts) + BN + ReLU -> sign
  conv4 3x3 s1 p1 (128->192, sign wts) + BN + ReLU -> sign
  conv5 1x1 s1 p0 (192->192, sign wts) + BN + ReLU
  conv6 1x1 (192->12) + b ; GAP ; FC 12x12 + b ; softmax

Device mapping:
  - Convs are shifted matmuls: activations live in SBUF as [C, Hp*Wp]
    (channel on partition, zero-padded spatial); each 3x3 tap is an
    accumulating matmul (K=Cin, M=Cout, N=outputs chunk).
  - sign(relu(bn(x))) == (x > t_c) per-channel (bn scale positive): one
    VectorE tensor_scalar is_gt PSUM->SBUF. Acts {0,1} and weights {-1,0,1}
    are exact in fp8e4m3, so layers 2-5 are exact integer arithmetic.
  - fp8 + DoubleRow packs 2 taps per matmul via the [Ki, 2, N] pair dim
    (arbitrary j-step in SBUF). L2 additionally packs 2 more taps on the
    partition dim via a second input copy shifted one row (K_eff=256).
  - conv6+GAP+FC folded: GAP via ScalarE activation accum_out; logits =
    (fc_w@conv6_w/1024) @ sums + (fc_w@conv6_b + fc_b) via one tiny matmul
    with a ones-row; softmax on device.
"""

import sys

sys.path.insert(0, "/opt/trn_rl_repo")

import numpy as np

import concourse.ap as apm
import concourse.bass as bass
import concourse.mybir as mybir
import concourse.bacc as bacc
import concourse.tile as tile
from concourse.bass_utils import run_bass_kernel_spmd

F32 = mybir.dt.float32
FP8 = mybir.dt.float8e4
AX = mybir.AxisListType
OP = mybir.AluOpType
ACT = mybir.ActivationFunctionType
DR = mybir.MatmulPerfMode.DoubleRow

N_CORES = 8
B = 16  # images per core

EPS = 1e-5

# geometry
H1, W1 = 64, 64          # conv1 output spatial
P1 = W1 + 2              # padded width/height for A1/A2 (66)
S1 = P1 * P1             # 4356
S1e = S1 + 8             # guard tail
BO1 = 4368               # A1 shifted-copy block offset (16-aligned)
H3, W3 = 32, 32          # conv3 output spatial
P3 = W3 + 2              # 34
S3 = P3 * P3             # 1156
S3e = S3 + 8
BO3 = 1168               # A3 shifted-copy block offset (16-aligned)
NPOS = H3 * W3           # 1024 valid positions for L5/GAP

_CACHE = {}


def _ap(base2d, off, dims):
    """Custom AP over an SBUF tile slice: base partition dim + free dims
    (supports overlapping patterns rearrange can't express)."""
    return apm.AP(tensor=base2d.tensor, offset=base2d.offset + off,
                  ap=[list(base2d.ap[0])] + [list(d) for d in dims])


def _build(reps=1):
    """Trace + compile the Bass program (cached). reps>1 replicates the whole
    pipeline on-device (for timing via wall-clock differencing)."""
    key = f"nc{reps}"
    if key in _CACHE:
        return _CACHE

    nc = bacc.Bacc("TRN2", target_bir_lowering=False, debug=False,
                   num_devices=N_CORES)

    # ---- DRAM I/O ----
    # host-side im2col of the 1-channel input: [B, 9, 64*64]
    dX = nc.dram_tensor("x", [B, 9, 4096], F32, kind="ExternalInput").ap()
    dW1T = nc.dram_tensor("w1t", [9, 64], F32, kind="ExternalInput").ap()
    dW2D1 = nc.dram_tensor("w2d1", [128, 256], FP8, kind="ExternalInput").ap()
    dW2D2 = nc.dram_tensor("w2d2", [128, 256], FP8, kind="ExternalInput").ap()
    dW2D3 = nc.dram_tensor("w2d3", [64, 256], FP8, kind="ExternalInput").ap()
    dW3 = nc.dram_tensor("w3", [128, 1152], FP8, kind="ExternalInput").ap()
    dW4DA = nc.dram_tensor("w4da", [128, 1024], FP8, kind="ExternalInput").ap()
    dW4SA = nc.dram_tensor("w4sa", [128, 128], FP8, kind="ExternalInput").ap()
    dW4DB = nc.dram_tensor("w4db", [128, 512], FP8, kind="ExternalInput").ap()
    dW4SB = nc.dram_tensor("w4sb", [128, 64], FP8, kind="ExternalInput").ap()
    dW5A = nc.dram_tensor("w5a", [128, 192], FP8, kind="ExternalInput").ap()
    dW5B = nc.dram_tensor("w5b", [64, 192], FP8, kind="ExternalInput").ap()
    dT1 = nc.dram_tensor("t1", [64, 1], F32, kind="ExternalInput").ap()
    dT2 = nc.dram_tensor("t2", [128, 1], F32, kind="ExternalInput").ap()
    dT3 = nc.dram_tensor("t3", [128, 1], F32, kind="ExternalInput").ap()
    dT4a = nc.dram_tensor("t4a", [128, 1], F32, kind="ExternalInput").ap()
    dT4b = nc.dram_tensor("t4b", [64, 1], F32, kind="ExternalInput").ap()
    dA5a = nc.dram_tensor("a5a", [128, 1], F32, kind="ExternalInput").ap()
    dA5b = nc.dram_tensor("a5b", [64, 1], F32, kind="ExternalInput").ap()
    dB5a = nc.dram_tensor("b5a", [128, 1], F32, kind="ExternalInput").ap()
    dB5b = nc.dram_tensor("b5b", [64, 1], F32, kind="ExternalInput").ap()
    dWTa = nc.dram_tensor("wta", [128, 12], F32, kind="ExternalInput").ap()
    dWTb = nc.dram_tensor("wtb", [65, 12], F32, kind="ExternalInput").ap()
    dY = nc.dram_tensor("y", [B, 12], F32, kind="ExternalOutput").ap()

    with tile.TileContext(nc) as tc:
        with tc.tile_pool(name="const", bufs=1) as cp, \
             tc.tile_pool(name="work", bufs=2) as wp, \
             tc.tile_pool(name="psum", bufs=3, space="PSUM") as pp:

            def ctile(name, shape, dtype):
                return cp.tile(shape, dtype, tag=name, name=name)

            # ---- persistent weight/param tiles ----
            cW1T = ctile("cW1T", [9, 64], F32)
            cW2D1 = ctile("cW2D1", [128, 256], FP8)
            cW2D2 = ctile("cW2D2", [128, 256], FP8)
            cW2D3 = ctile("cW2D3", [64, 256], FP8)
            cW3 = ctile("cW3", [128, 1152], FP8)
            cW4DA = ctile("cW4DA", [128, 1024], FP8)
            cW4SA = ctile("cW4SA", [128, 128], FP8)
            cW4DB = ctile("cW4DB", [128, 512], FP8)
            cW4SB = ctile("cW4SB", [128, 64], FP8)
            cW5A = ctile("cW5A", [128, 192], FP8)
            cW5B = ctile("cW5B", [64, 192], FP8)
            cT1 = ctile("cT1", [64, 1], F32)
            cT2 = ctile("cT2", [128, 1], F32)
            cT3 = ctile("cT3", [128, 1], F32)
            cT4a = ctile("cT4a", [128, 1], F32)
            cT4b = ctile("cT4b", [64, 1], F32)
            cA5a = ctile("cA5a", [128, 1], F32)
            cA5b = ctile("cA5b", [64, 1], F32)
            cB5a = ctile("cB5a", [128, 1], F32)
            cB5b = ctile("cB5b", [64, 1], F32)
            cWTa = ctile("cWTa", [128, 12], F32)
            cWTb = ctile("cWTb", [65, 12], F32)

            for t_, d_ in [(cW1T, dW1T), (cW2D1, dW2D1), (cW2D2, dW2D2),
                           (cW3, dW3), (cW4DA, dW4DA), (cW4SA, dW4SA),
                           (cW4DB, dW4DB), (cW4SB, dW4SB), (cW5A, dW5A),
                           (cW5B, dW5B), (cT1, dT1), (cT2, dT2), (cT3, dT3),
                           (cT4a, dT4a), (cT4b, dT4b), (cA5a, dA5a),
                           (cA5b, dA5b), (cB5a, dB5a), (cB5b, dB5b),
                           (cWTa, dWTa), (cWTb, dWTb)]:
                nc.sync.dma_start(t_[:], d_[:])
            nc.sync.dma_start(cW2D3[:], dW2D3[:])

            # ---- persistent activation buffers (double-buffered by parity) ---
            IC = [ctile(f"IC{p}", [9, 4096], F32) for p in range(2)]
            # A1/A3 carry 3 blocks: +0, +1 col, +row-pair tap shift (130/32),
            # at 16-aligned block offsets for DoubleRow's pair-dim step.
            A1 = [ctile(f"A1_{p}", [128, 3 * BO1], FP8) for p in range(2)]
            A2 = [ctile(f"A2_{p}", [128, S1e], FP8) for p in range(2)]
            A3 = [ctile(f"A3_{p}", [128, 3 * BO3], FP8) for p in range(2)]
            A4a = [ctile(f"A4a_{p}", [128, NPOS], FP8) for p in range(2)]
            A4b = [ctile(f"A4b_{p}", [64, NPOS], FP8) for p in range(2)]
            MACCa = ctile("MACCa", [128, 2 * B], F32)
            MACCb = ctile("MACCb", [64, 2 * B], F32)
            Msum = ctile("Msum", [128, B], F32)
            MsumB = ctile("MsumB", [65, B], F32)

            # zero padding once; interiors are rewritten every image
            for p in range(2):
                nc.gpsimd.memset(A1[p][:], 0.0)
                nc.gpsimd.memset(A2[p][:], 0.0)
                nc.gpsimd.memset(A3[p][:], 0.0)
            nc.vector.memset(MsumB[64:65, :], 1.0)

            def a1v(p):
                return A1[p][:, 0:S1].rearrange("p (a b) -> p a b", b=P1)

            def a3v(p):
                return A3[p][:, 0:S3].rearrange("p (a b) -> p a b", b=P3)

            w2d1 = cW2D1[:].rearrange("p (j m) -> p j m", j=2)
            w2d2 = cW2D2[:].rearrange("p (j m) -> p j m", j=2)
            w2d3 = cW2D3[:].rearrange("p (j m) -> p j m", j=2)

            # L2 row-aligned chunks (valid-only strided epilogue)
            L2CH = [(r * 7, 7) for r in range(9)] + [(63, 1)]
            # L4 row-aligned chunks (A4 is unpadded)
            L4CH = [(0, 15), (15, 15), (30, 2)]
            # L4 DoubleRow tap pairs -> (base offset, j block step)
            # deltas: +1 -> blk1 (BO3), +32 -> blk2 (2*BO3)
            L4P = [(0, BO3), (2, 2 * BO3), (35, BO3), (68, BO3)]

            # ---------------- conv1 + binarize -> A1 (both copies) ------------
            def conv1_block(i):
                p = i % 2
                nc.sync.dma_start(IC[p][:], dX[i])
                for r in range(8):
                    ps = pp.tile([64, 512], F32, tag="mm", name=f"ps_c1_{i}_{r}")
                    nc.tensor.matmul(
                        ps[:], cW1T[:], IC[p][0:9, r * 512:(r + 1) * 512],
                        start=True, stop=True)
                    nc.vector.tensor_scalar(
                        a1v(p)[0:64, r * 8 + 1: r * 8 + 9, 1:65],
                        ps[:].rearrange("q (a b) -> q a b", b=64),
                        cT1[:], None, OP.is_gt)
                # copy2 at partitions 64..127, shifted down one row:
                # copy2[q] = copy1[q + P1]
                nc.sync.dma_start(A1[p][64:128, 0:S1 - P1], A1[p][0:64, P1:S1])
                # shifted blocks for DoubleRow pair dim (both partition halves)
                nc.sync.dma_start(A1[p][:, BO1:BO1 + 4360], A1[p][:, 1:4361])
                nc.sync.dma_start(A1[p][:, 2 * BO1:2 * BO1 + 4232],
                                  A1[p][:, 130:4362])

            # ---------------- layer bodies ------------------------------------
            def l2_block(i):
                # out(q=y*66+x) needs img[q + ky*66 + kx]; copy1(lo)=img[q],
                # copy2(hi)=img[q+66] => partition half packs ky,ky+1;
                # j blocks: blk1[q]=img[q+1], blk2[q]=img[q+130].
                p = i % 2
                a1f = A1[p][:, :]
                a1lo = A1[p][0:64, :]
                a2vv = A2[p][:, 0:S1].rearrange("p (a b) -> p a b", b=P1)
                for ci, (y0, nr) in enumerate(L2CH):
                    q0 = y0 * P1
                    n = nr * P1
                    ps = pp.tile([128, 512], F32, tag="mm", name=f"ps_l2_{i}_{ci}")
                    # taps (0,0),(1,0) | (0,1),(1,1)
                    nc.tensor.matmul(
                        ps[:, 0:n], w2d1,
                        _ap(a1f, q0, [[BO1, 2], [1, n]]),
                        start=True, stop=False, perf_mode=DR)
                    # taps (0,2),(1,2) | (2,0),zero
                    nc.tensor.matmul(
                        ps[:, 0:n], w2d2,
                        _ap(a1f, q0 + 2, [[2 * BO1, 2], [1, n]]),
                        start=False, stop=False, perf_mode=DR)
                    # taps (2,1) | (2,2)  (copy1 partitions, +2 rows + 1 base)
                    nc.tensor.matmul(
                        ps[:, 0:n], w2d3,
                        _ap(a1lo, q0 + 2 * P1 + 1, [[BO1, 2], [1, n]]),
                        start=False, stop=True, perf_mode=DR)
                    nc.vector.tensor_scalar(
                        a2vv[:, y0 + 1:y0 + 1 + nr, 1:65],
                        ps[:, 0:n].rearrange("q (a b) -> q a b", b=P1)
                        [:, :, 0:64],
                        cT2[:], None, OP.is_gt)

            def l3_block(i):
                p = i % 2
                a2v = A2[p][:, 0:S1].rearrange("p (a b) -> p a b", b=P1)
                for r in range(2):
                    y0 = r * 16
                    ps = pp.tile([128, 512], F32, tag="mm", name=f"ps_l3_{i}_{r}")
                    psv = ps[:].rearrange("q (a b) -> q a b", b=32)
                    for t in range(9):
                        ky, kx = t // 3, t % 3
                        nc.tensor.matmul(
                            psv,
                            cW3[:, t * 128:(t + 1) * 128],
                            a2v[:, 2 * y0 + ky: 2 * y0 + ky + 31: 2,
                                kx: kx + 63: 2],
                            start=(t == 0), stop=(t == 8))
                    nc.vector.tensor_scalar(
                        a3v(p)[:, y0 + 1:y0 + 17, 1:33], psv,
                        cT3[:], None, OP.is_gt)
                # shifted blocks for L4's DoubleRow pair dim
                nc.sync.dma_start(A3[p][:, BO3:BO3 + 1160], A3[p][:, 1:1161])
                nc.sync.dma_start(A3[p][:, 2 * BO3:2 * BO3 + 1128],
                                  A3[p][:, 32:1160])

            def l4_block(i):
                p = i % 2
                a3f = A3[p][:, :]
                a4av = A4a[p][:, :].rearrange("p (a b) -> p a b", b=32)
                a4bv = A4b[p][:, :].rearrange("p (a b) -> p a b", b=32)
                for ci, (y0, nr) in enumerate(L4CH):
                    q0 = y0 * P3
                    n = nr * P3
                    psa = pp.tile([128, 512], F32, tag="mb", bufs=2,
                                  name=f"ps_l4a_{i}_{ci}")
                    psb = pp.tile([64, 512], F32, tag="mb2", bufs=2,
                                  name=f"ps_l4b_{i}_{ci}")
                    for mb, (psx, wd, ws, mw) in enumerate(
                            [(0, cW4DA, cW4SA, 128), (1, cW4DB, cW4SB, 64)]):
                        psx = psa if mb == 0 else psb
                        wd = cW4DA if mb == 0 else cW4DB
                        ws = cW4SA if mb == 0 else cW4SB
                        mw = 128 if mb == 0 else 64
                        for pi, (oa, js) in enumerate(L4P):
                            nc.tensor.matmul(
                                psx[:, 0:n],
                                wd[:, pi * 2 * mw:(pi + 1) * 2 * mw]
                                .rearrange("p (j m) -> p j m", j=2),
                                _ap(a3f, q0 + oa, [[js, 2], [1, n]]),
                                start=(pi == 0), stop=False, perf_mode=DR)
                        nc.tensor.matmul(
                            psx[:, 0:n], ws[:, 0:mw],
                            A3[p][:, q0 + 2 * P3 + 2: q0 + 2 * P3 + 2 + n],
                            start=False, stop=True)
                    nc.vector.tensor_scalar(
                        a4av[:, y0:y0 + nr, 0:32],
                        psa[:, 0:n].rearrange("q (a b) -> q a b", b=P3)
                        [:, :, 0:32],
                        cT4a[:], None, OP.is_gt)
                    nc.vector.tensor_scalar(
                        a4bv[:, y0:y0 + nr, 0:32],
                        psb[:, 0:n].rearrange("q (a b) -> q a b", b=P3)
                        [:, :, 0:32],
                        cT4b[:], None, OP.is_gt)

            def l5_block(i):
                p = i % 2
                for c in range(2):
                    sl = slice(c * 512, (c + 1) * 512)
                    psa = pp.tile([128, 512], F32, tag="mb", bufs=2,
                                  name=f"ps_l5a_{i}_{c}")
                    psb = pp.tile([64, 512], F32, tag="mb2", bufs=2,
                                  name=f"ps_l5b_{i}_{c}")
                    nc.tensor.matmul(psa[:], cW5A[:, 0:128], A4a[p][:, sl],
                                     start=True, stop=False)
                    nc.tensor.matmul(psa[:], cW5B[:, 0:128], A4b[p][:, sl],
                                     start=False, stop=True)
                    nc.tensor.matmul(psb[:], cW5A[:, 128:192], A4a[p][:, sl],
                                     start=True, stop=False)
                    nc.tensor.matmul(psb[:], cW5B[:, 128:192], A4b[p][:, sl],
                                     start=False, stop=True)
                    # h5 = relu(a5*conv + b5); GAP partial sums via accum_out
                    scra = wp.tile([128, 512], F32, tag="scr_a",
                                   name=f"scra_{i}_{c}")
                    scrb = wp.tile([64, 512], F32, tag="scr_b",
                                   name=f"scrb_{i}_{c}")
                    nc.scalar.activation(
                        scra[:], psa[:], ACT.Relu, bias=cB5a[:], scale=cA5a[:],
                        accum_out=MACCa[:, 2 * i + c: 2 * i + c + 1])
                    nc.scalar.activation(
                        scrb[:], psb[:], ACT.Relu, bias=cB5b[:], scale=cA5b[:],
                        accum_out=MACCb[:, 2 * i + c: 2 * i + c + 1])

            # ---------------- main pipeline -----------------------------------
            for _rep in range(reps):
                conv1_block(0)
                for i in range(B):
                    if i + 1 < B:
                        conv1_block(i + 1)
                    l2_block(i)
                    l3_block(i)
                    l4_block(i)
                    l5_block(i)

            # ---------------- GAP/FC/softmax tail -----------------------------
            nc.vector.tensor_reduce(
                Msum[:, 0:B], MACCa[:].rearrange("p (i c) -> p i c", c=2),
                axis=AX.X, op=OP.add)
            nc.vector.tensor_reduce(
                MsumB[0:64, 0:B], MACCb[:].rearrange("p (i c) -> p i c", c=2),
                axis=AX.X, op=OP.add)

            psf = pp.tile([16, 12], F32, tag="fc", bufs=1, name="ps_fc")
            nc.tensor.matmul(psf[:], Msum[:, 0:B], cWTa[:],
                             start=True, stop=False)
            nc.tensor.matmul(psf[:], MsumB[:, 0:B], cWTb[:],
                             start=False, stop=True)

            negmax = cp.tile([16, 1], F32, tag="negmax", name="negmax")
            esum = cp.tile([16, 1], F32, tag="esum", name="esum")
            rsum = cp.tile([16, 1], F32, tag="rsum", name="rsum")
            etile = cp.tile([16, 12], F32, tag="etile", name="etile")
            yout = cp.tile([16, 12], F32, tag="yout", name="yout")

            nc.vector.tensor_reduce(negmax[:], psf[:], axis=AX.X, op=OP.max,
                                    negate=True)
            nc.scalar.activation(etile[:], psf[:], ACT.Exp, bias=negmax[:],
                                 scale=1.0, accum_out=esum[:])
            nc.vector.reciprocal(rsum[:], esum[:])
            nc.vector.tensor_scalar(yout[:], etile[:], rsum[:], None, OP.mult)
            nc.sync.dma_start(dY[:], yout[:])

    nc.compile()
    _CACHE[key] = nc
    return _CACHE


def _host_prep(inputs):
    """Fold BN into thresholds/affines; sign-binarize weights; build per-core
    input maps."""
    f32 = np.float32
    fp8 = mybir.dt.np(FP8)

    x = np.asarray(inputs["x"], f32)

    def inv(l):
        return (np.asarray(inputs[f"bn{l}_g"], f32)
                / np.sqrt(np.asarray(inputs[f"bn{l}_v"], f32) + np.float32(EPS)))

    invs = {l: inv(l) for l in (1, 2, 3, 4, 5)}
    for l in (1, 2, 3, 4):
        assert (invs[l] > 0).all(), f"bn{l} scale not positive"

    def thr(l):
        return (np.asarray(inputs[f"bn{l}_m"], f32)
                - np.asarray(inputs[f"bn{l}_b"], f32) / invs[l])

    t1 = (thr(1) - np.asarray(inputs["conv1_b"], f32)).reshape(64, 1)
    t2 = thr(2).reshape(128, 1)
    t3 = thr(3).reshape(128, 1)
    t4 = thr(4)
    a5 = invs[5]
    b5 = (np.asarray(inputs["bn5_b"], f32)
          - np.asarray(inputs["bn5_m"], f32) * invs[5])

    # conv1 weights -> lhsT [tap, cout]
    w1 = np.asarray(inputs["conv1_w"], f32)           # [64,1,3,3]
    w1t = np.ascontiguousarray(w1[:, 0].reshape(64, 9).T)  # [9, 64]

    sw2 = np.sign(np.asarray(inputs["w2"], f32))       # [128,64,3,3]
    sw3 = np.sign(np.asarray(inputs["w3"], f32))       # [128,128,3,3]
    sw4 = np.sign(np.asarray(inputs["w4"], f32))       # [192,128,3,3]
    sw5 = np.sign(np.asarray(inputs["w5"], f32))       # [192,192,1,1]

    # L2 DoubleRow packs: partitions = [ci(64) x ky-half], j = second tap dim
    # MM1: j -> kx in {0,1} over ky-halves {0,1}
    w2d1 = np.zeros((128, 2, 128), f32)
    for h in range(2):
        for j in range(2):
            w2d1[64 * h:64 * (h + 1), j] = sw2[:, :, h, j].T
    # MM2: j0 -> (ky=h, kx=2); j1 -> (2,0) on lo half, zero on hi half
    w2d2 = np.zeros((128, 2, 128), f32)
    for h in range(2):
        w2d2[64 * h:64 * (h + 1), 0] = sw2[:, :, h, 2].T
    w2d2[0:64, 1] = sw2[:, :, 2, 0].T
    # MM3 (lo partitions only): j0 -> (2,1); j1 -> (2,2)
    w2d3 = np.zeros((64, 2, 128), f32)
    w2d3[:, 0] = sw2[:, :, 2, 1].T
    w2d3[:, 1] = sw2[:, :, 2, 2].T

    w3t = np.zeros((128, 1152), f32)
    for t in range(9):
        w3t[:, t * 128:(t + 1) * 128] = sw3[:, :, t // 3, t % 3].T

    # L4 DoubleRow pairs + solo (2,2)
    L4P = [((0, 0), (0, 1)), ((0, 2), (1, 0)), ((1, 1), (1, 2)),
           ((2, 0), (2, 1))]
    w4da = np.zeros((128, 4, 2, 128), f32)
    w4db = np.zeros((128, 4, 2, 64), f32)
    for pi, (ta, tb) in enumerate(L4P):
        for j, (ky, kx) in enumerate((ta, tb)):
            w4da[:, pi, j] = sw4[:128, :, ky, kx].T
            w4db[:, pi, j] = sw4[128:, :, ky, kx].T
    w4sa = np.ascontiguousarray(sw4[:128, :, 2, 2].T)   # [128, 128]
    w4sb = np.ascontiguousarray(sw4[128:, :, 2, 2].T)   # [128, 64]

    w5 = sw5[:, :, 0, 0]                               # [co=192, ci=192]
    w5a = np.ascontiguousarray(w5[:, :128].T)          # [128, 192]
    w5b = np.ascontiguousarray(w5[:, 128:].T)          # [64, 192]

    fc_w = np.asarray(inputs["fc_w"], f32)
    c6w = np.asarray(inputs["conv6_w"], f32)[:, :, 0, 0]   # [12, 192]
    Wp = (fc_w @ c6w) / np.float32(NPOS)               # [12, 192]
    cvec = fc_w @ np.asarray(inputs["conv6_b"], f32) + np.asarray(
        inputs["fc_b"], f32)                           # [12]
    wta = np.ascontiguousarray(Wp[:, :128].T)          # [128, 12]
    wtb = np.zeros((65, 12), f32)
    wtb[:64] = Wp[:, 128:].T
    wtb[64] = cvec

    shared = {
        "w1t": w1t.astype(f32),
        "w2d1": w2d1.reshape(128, 256).astype(fp8),
        "w2d2": w2d2.reshape(128, 256).astype(fp8),
        "w2d3": w2d3.reshape(64, 256).astype(fp8),
        "w3": w3t.astype(fp8),
        "w4da": w4da.reshape(128, 1024).astype(fp8),
        "w4sa": w4sa.astype(fp8),
        "w4db": w4db.reshape(128, 512).astype(fp8),
        "w4sb": w4sb.astype(fp8),
        "w5a": w5a.astype(fp8), "w5b": w5b.astype(fp8),
        "t1": t1.astype(f32), "t2": t2.astype(f32), "t3": t3.astype(f32),
        "t4a": t4[:128].reshape(128, 1).astype(f32),
        "t4b": t4[128:].reshape(64, 1).astype(f32),
        "a5a": a5[:128].reshape(128, 1).astype(f32),
        "a5b": a5[128:].reshape(64, 1).astype(f32),
        "b5a": b5[:128].reshape(128, 1).astype(f32),
        "b5b": b5[128:].reshape(64, 1).astype(f32),
        "wta": wta.astype(f32), "wtb": wtb.astype(f32),
    }
    # host im2col: cols[b, 3*ky+kx, y*64+x] = xpad[b, 2y+ky, 2x+kx]
    xpad = np.pad(x[:, 0], ((0, 0), (1, 1), (1, 1)))
    cols = np.stack([xpad[:, ky:ky + 127:2, kx:kx + 127:2]
                     for ky in range(3) for kx in range(3)],
                    axis=1).reshape(x.shape[0], 9, 4096)
    in_maps = []
    for c in range(N_CORES):
        m = dict(shared)
        m["x"] = np.ascontiguousarray(cols[c * B:(c + 1) * B])
        in_maps.append(m)
    return in_maps


def kernel(**inputs):
    cache = _build()
    in_maps = _host_prep(inputs)
    res = run_bass_kernel_spmd(cache["nc1"], in_maps,
                               core_ids=list(range(N_CORES)))
    _CACHE["last_results"] = res
    return np.concatenate([res.results[c]["y"] for c in range(N_CORES)],
                          axis=0)


# ---------------------------------------------------------------------------
# numpy golden model of the device algorithm (for fast validation in test.py)
# ---------------------------------------------------------------------------
def golden(inputs):
    f32 = np.float32
    in_maps = _host_prep(inputs)
    outs = []
    for m in in_maps:
        cols = np.asarray(m["x"], f32)  # [B, 9, 4096] host im2col
        t1 = m["t1"][:, 0]
        c1 = np.einsum("btn,tc->bcn", cols, m["w1t"]).reshape(-1, 64, 64, 64)
        a1 = (c1 > t1[None, :, None, None]).astype(f32)

        def bconv(a, wt, stride):
            # a: [B,C,H,W] binary; wt[ci, t, co] for t = 3*ky+kx
            Bn, C, H, W = a.shape
            ap = np.pad(a, ((0, 0), (0, 0), (1, 1), (1, 1)))
            Ho, Wo = H // stride, W // stride
            out = np.zeros((Bn, wt.shape[2], Ho, Wo), f32)
            for t in range(9):
                ky, kx = t // 3, t % 3
                sl = ap[:, :, ky:ky + H:stride, kx:kx + W:stride][
                    :, :, :Ho, :Wo]
                out += np.einsum("bcyx,cd->bdyx", sl, wt[:, t])
            return out

        # reconstruct w2 [ci, t, co] from the DR packs
        w2d1 = np.asarray(m["w2d1"], f32).reshape(128, 2, 128)
        w2d2 = np.asarray(m["w2d2"], f32).reshape(128, 2, 128)
        w2d3 = np.asarray(m["w2d3"], f32).reshape(64, 2, 128)
        w2 = np.zeros((64, 9, 128), f32)
        for h in range(2):
            for j in range(2):
                w2[:, 3 * h + j] = w2d1[64 * h:64 * h + 64, j]
            w2[:, 3 * h + 2] = w2d2[64 * h:64 * h + 64, 0]
        w2[:, 6] = w2d2[0:64, 1]
        w2[:, 7] = w2d3[:, 0]
        w2[:, 8] = w2d3[:, 1]
        c2 = bconv(a1, w2, 1)
        a2 = (c2 > m["t2"].reshape(1, 128, 1, 1)).astype(f32)

        w3 = np.asarray(m["w3"], f32).reshape(128, 9, 128)
        c3 = bconv(a2, w3, 2)
        a3 = (c3 > m["t3"].reshape(1, 128, 1, 1)).astype(f32)

        # reconstruct w4 [ci, t, co] from DR packs + solo
        L4P = [((0, 0), (0, 1)), ((0, 2), (1, 0)), ((1, 1), (1, 2)),
               ((2, 0), (2, 1))]
        w4da = np.asarray(m["w4da"], f32).reshape(128, 4, 2, 128)
        w4db = np.asarray(m["w4db"], f32).reshape(128, 4, 2, 64)
        w4 = np.zeros((128, 9, 192), f32)
        for pi, (ta, tb) in enumerate(L4P):
            for j, (ky, kx) in enumerate((ta, tb)):
                w4[:, 3 * ky + kx, :128] = w4da[:, pi, j]
                w4[:, 3 * ky + kx, 128:] = w4db[:, pi, j]
        w4[:, 8, :128] = np.asarray(m["w4sa"], f32)
        w4[:, 8, 128:] = np.asarray(m["w4sb"], f32)
        c4 = bconv(a3, w4, 1)
        a4 = np.concatenate([
            (c4[:, :128] > m["t4a"].reshape(1, 128, 1, 1)).astype(f32),
            (c4[:, 128:] > m["t4b"].reshape(1, 64, 1, 1)).astype(f32)], axis=1)

        w5 = np.concatenate([np.asarray(m["w5a"], f32),
                             np.asarray(m["w5b"], f32)], axis=0)  # [192, 192]
        c5 = np.einsum("bcyx,cd->bdyx", a4, w5)
        a5v = np.concatenate([m["a5a"], m["a5b"]], axis=0).reshape(1, 192, 1, 1)
        b5v = np.concatenate([m["b5a"], m["b5b"]], axis=0).reshape(1, 192, 1, 1)
        h5 = np.maximum(a5v * c5 + b5v, 0.0)
        sums = h5.sum(axis=(2, 3))  # [B, 192]
        WT = np.concatenate([m["wta"], m["wtb"][:64]], axis=0)  # [192, 12]
        logits = sums @ WT + m["wtb"][64][None, :]
        z = logits - logits.max(axis=1, keepdims=True)
        e = np.exp(z)
        outs.append(e / e.sum(axis=1, keepdims=True))
    return np.concatenate(outs, axis=0)


# revision 47
# speedup vs baseline: 3.8751x; 1.1845x over previous
"""Trainium2 Bass kernel for BinarizedInputNetwork.

Contract: kernel(**inputs) takes the FULL unsharded inputs (batch 128) and
returns the FULL [128, 12] float32 softmax output. Internally shards the
batch across 8 NeuronCores (16 images each), runs one SPMD Bass program.

Network (per image, input [1,128,128]):
  conv1 3x3 s2 p1 (1->64)  + BN + ReLU -> sign       => binary acts {0,1}
  conv2 3x3 s1 p1 (64->128, sign wts)  + BN + ReLU -> sign
  conv3 3x3 s2 p1 (128->128, sign wts) + BN + ReLU -> sign
  conv4 3x3 s1 p1 (128->192, sign wts) + BN + ReLU -> sign
  conv5 1x1 s1 p0 (192->192, sign wts) + BN + ReLU
  conv6 1x1 (192->12) + b ; GAP ; FC 12x12 + b ; softmax

Device mapping:
  - Convs are shifted matmuls: activations live in SBUF as [C, Hp*Wp]
    (channel on partition, zero-padded spatial); each 3x3 tap is an
    accumulating matmul (K=Cin, M=Cout, N=outputs chunk).
  - sign(relu(bn(x))) == (x > t_c) per-channel (bn scale positive): one
    VectorE tensor_scalar is_gt PSUM->SBUF. Acts {0,1} and weights {-1,0,1}
    are exact in fp8e4m3, so layers 2-5 are exact integer arithmetic.
  - fp8 + DoubleRow packs 2 taps per matmul via the [Ki, 2, N] pair dim
    (arbitrary j-step in SBUF). L2 additionally packs 2 more taps on the
    partition dim via a second input copy shifted one row (K_eff=256).
  - conv6+GAP+FC folded: GAP via ScalarE activation accum_out; logits =
    (fc_w@conv6_w/1024) @ sums + (fc_w@conv6_b + fc_b) via one tiny matmul
    with a ones-row; softmax on device.
"""

import sys

sys.path.insert(0, "/opt/trn_rl_repo")

import numpy as np

import concourse.ap as apm
import concourse.bass as bass
import concourse.mybir as mybir
import concourse.bacc as bacc
import concourse.tile as tile
from concourse.bass_utils import run_bass_kernel_spmd

F32 = mybir.dt.float32
FP8 = mybir.dt.float8e4
AX = mybir.AxisListType
OP = mybir.AluOpType
ACT = mybir.ActivationFunctionType
DR = mybir.MatmulPerfMode.DoubleRow

N_CORES = 8
B = 16  # images per core

EPS = 1e-5

# geometry
H1, W1 = 64, 64          # conv1 output spatial
P1 = W1 + 2              # padded width/height for A1/A2 (66)
S1 = P1 * P1             # 4356
S1e = S1 + 8             # guard tail
BO1 = 4368               # A1 shifted-copy block offset (16-aligned)
H3, W3 = 32, 32          # conv3 output spatial
P3 = W3 + 2              # 34
S3 = P3 * P3             # 1156
S3e = S3 + 8
BO3 = 1168               # A3 shifted-copy block offset (16-aligned)
NPOS = H3 * W3           # 1024 valid positions for L5/GAP

_CACHE = {}


def _ap(base2d, off, dims):
    """Custom AP over an SBUF tile slice: base partition dim + free dims
    (supports overlapping patterns rearrange can't express)."""
    return apm.AP(tensor=base2d.tensor, offset=base2d.offset + off,
                  ap=[list(base2d.ap[0])] + [list(d) for d in dims])


def _build(reps=1):
    """Trace + compile the Bass program (cached). reps>1 replicates the whole
    pipeline on-device (for timing via wall-clock differencing)."""
    key = f"nc{reps}"
    if key in _CACHE:
        return _CACHE

    nc = bacc.Bacc("TRN2", target_bir_lowering=False, debug=False,
                   num_devices=N_CORES)

    # ---- DRAM I/O ----
    # host-side im2col of the 1-channel input: [B, 9, 64*64]
    dX = nc.dram_tensor("x", [B, 9, 4096], F32, kind="ExternalInput").ap()
    # conv1 weights at partition rows 0-8 and 64-72 (2-way row tiling)
    dW1T = nc.dram_tensor("w1t4", [128, 64], F32, kind="ExternalInput").ap()
    dW2D1 = nc.dram_tensor("w2d1", [128, 256], FP8, kind="ExternalInput").ap()
    dW2D2 = nc.dram_tensor("w2d2", [128, 256], FP8, kind="ExternalInput").ap()
    dW2D3 = nc.dram_tensor("w2d3", [64, 256], FP8, kind="ExternalInput").ap()
    dW3D = nc.dram_tensor("w3d", [128, 768], FP8, kind="ExternalInput").ap()
    dW3D3 = nc.dram_tensor("w3d3", [128, 256], FP8, kind="ExternalInput").ap()
    dW3S = nc.dram_tensor("w3s", [128, 128], FP8, kind="ExternalInput").ap()
    dW4DA = nc.dram_tensor("w4da", [128, 1024], FP8, kind="ExternalInput").ap()
    dW4SA = nc.dram_tensor("w4sa", [128, 128], FP8, kind="ExternalInput").ap()
    dW4DB = nc.dram_tensor("w4db", [128, 512], FP8, kind="ExternalInput").ap()
    dW4SB = nc.dram_tensor("w4sb", [128, 64], FP8, kind="ExternalInput").ap()
    dW5DA = nc.dram_tensor("w5da", [128, 256], FP8, kind="ExternalInput").ap()
    dW5DB = nc.dram_tensor("w5db", [128, 128], FP8, kind="ExternalInput").ap()
    dT1 = nc.dram_tensor("t1", [64, 1], F32, kind="ExternalInput").ap()
    dT2 = nc.dram_tensor("t2", [128, 1], F32, kind="ExternalInput").ap()
    dT3 = nc.dram_tensor("t3", [128, 1], F32, kind="ExternalInput").ap()
    dT4a = nc.dram_tensor("t4a", [128, 1], F32, kind="ExternalInput").ap()
    dT4b = nc.dram_tensor("t4b", [64, 1], F32, kind="ExternalInput").ap()
    dA5a = nc.dram_tensor("a5a", [128, 1], F32, kind="ExternalInput").ap()
    dA5b = nc.dram_tensor("a5b", [64, 1], F32, kind="ExternalInput").ap()
    dB5a = nc.dram_tensor("b5a", [128, 1], F32, kind="ExternalInput").ap()
    dB5b = nc.dram_tensor("b5b", [64, 1], F32, kind="ExternalInput").ap()
    dWTa = nc.dram_tensor("wta", [128, 12], F32, kind="ExternalInput").ap()
    dWTb = nc.dram_tensor("wtb", [65, 12], F32, kind="ExternalInput").ap()
    dY = nc.dram_tensor("y", [B, 12], F32, kind="ExternalOutput").ap()

    with tile.TileContext(nc) as tc:
        with tc.tile_pool(name="const", bufs=1) as cp, \
             tc.tile_pool(name="work", bufs=2) as wp, \
             tc.tile_pool(name="psum", bufs=3, space="PSUM") as pp:

            def ctile(name, shape, dtype):
                return cp.tile(shape, dtype, tag=name, name=name)

            # ---- persistent weight/param tiles ----
            cW1T = ctile("cW1T", [128, 64], F32)
            cW2D1 = ctile("cW2D1", [128, 256], FP8)
            cW2D2 = ctile("cW2D2", [128, 256], FP8)
            cW2D3 = ctile("cW2D3", [64, 256], FP8)
            cW3D = ctile("cW3D", [128, 768], FP8)
            cW3D3 = ctile("cW3D3", [128, 256], FP8)
            cW3S = ctile("cW3S", [128, 128], FP8)
            cW4DA = ctile("cW4DA", [128, 1024], FP8)
            cW4SA = ctile("cW4SA", [128, 128], FP8)
            cW4DB = ctile("cW4DB", [128, 512], FP8)
            cW4SB = ctile("cW4SB", [128, 64], FP8)
            cW5DA = ctile("cW5DA", [128, 256], FP8)
            cW5DB = ctile("cW5DB", [128, 128], FP8)
            cT1 = ctile("cT1", [64, 1], F32)
            cT2 = ctile("cT2", [128, 1], F32)
            cT3 = ctile("cT3", [128, 1], F32)
            cT4a = ctile("cT4a", [128, 1], F32)
            cT4b = ctile("cT4b", [64, 1], F32)
            cA5a = ctile("cA5a", [128, 1], F32)
            cA5b = ctile("cA5b", [64, 1], F32)
            cB5a = ctile("cB5a", [128, 1], F32)
            cB5b = ctile("cB5b", [64, 1], F32)
            cWTa = ctile("cWTa", [128, 12], F32)
            cWTb = ctile("cWTb", [65, 12], F32)

            for t_, d_ in [(cW1T, dW1T), (cW2D1, dW2D1), (cW2D2, dW2D2),
                           (cW3D, dW3D), (cW3D3, dW3D3), (cW3S, dW3S),
                           (cW4DA, dW4DA), (cW4SA, dW4SA),
                           (cW4DB, dW4DB), (cW4SB, dW4SB), (cW5DA, dW5DA),
                           (cW5DB, dW5DB), (cT1, dT1), (cT2, dT2), (cT3, dT3),
                           (cT4a, dT4a), (cT4b, dT4b), (cA5a, dA5a),
                           (cA5b, dA5b), (cB5a, dB5a), (cB5b, dB5b),
                           (cWTa, dWTa), (cWTb, dWTb)]:
                nc.sync.dma_start(t_[:], d_[:])
            nc.sync.dma_start(cW2D3[:], dW2D3[:])

            # ---- persistent activation buffers (double-buffered by parity) ---
            # IC: image pair (g=0 at partitions 0-8, g=1 at 64-72)
            IC = [ctile(f"IC{p}", [128, 4096], F32) for p in range(2)]
            # A1/A2/A3 carry shifted-copy blocks at 16-aligned offsets for
            # DoubleRow's pair-dim step. A1: {+0, +1, +130}; A2: {+0, +66, +1};
            # A3: {+0, +1, +32}.
            A1 = [ctile(f"A1_{p}", [128, 3 * BO1], FP8) for p in range(4)]
            A2 = [ctile(f"A2_{p}", [128, 3 * BO1], FP8) for p in range(2)]
            A3 = [ctile(f"A3_{p}", [128, 3 * BO3], FP8) for p in range(2)]
            # A4: block0 = channels 0-127; block1 (offset NPOS) = channels
            # 128-191 on partitions 0-63, zeros above
            A4 = [ctile(f"A4_{p}", [128, 2 * NPOS], FP8) for p in range(2)]
            MACCa = ctile("MACCa", [128, 2 * B], F32)
            MACCb = ctile("MACCb", [64, 2 * B], F32)
            Msum = ctile("Msum", [128, B], F32)
            MsumB = ctile("MsumB", [65, B], F32)

            # zero padding once; interiors are rewritten every image
            for p in range(4):
                nc.gpsimd.memset(A1[p][:], 0.0)
            for p in range(2):
                nc.gpsimd.memset(A2[p][:], 0.0)
                nc.gpsimd.memset(A3[p][:], 0.0)
                nc.gpsimd.memset(A4[p][64:128, NPOS:2 * NPOS], 0.0)
            nc.vector.memset(MsumB[64:65, :], 1.0)

            def a1v(p):
                return A1[p][:, 0:S1].rearrange("p (a b) -> p a b", b=P1)

            def a3v(p):
                return A3[p][:, 0:S3].rearrange("p (a b) -> p a b", b=P3)

            w2d1 = cW2D1[:].rearrange("p (j m) -> p j m", j=2)
            w2d2 = cW2D2[:].rearrange("p (j m) -> p j m", j=2)
            w2d3 = cW2D3[:].rearrange("p (j m) -> p j m", j=2)

            # L2 row-aligned chunks (valid-only strided epilogue)
            L2CH = [(r * 7, 7) for r in range(9)] + [(63, 1)]
            # L4 row-aligned chunks (A4 is unpadded)
            L4CH = [(0, 15), (15, 15), (30, 2)]
            # L4 DoubleRow tap pairs -> (base offset, j block step)
            # deltas: +1 -> blk1 (BO3), +32 -> blk2 (2*BO3)
            L4P = [(0, BO3), (2, 2 * BO3), (35, BO3), (68, BO3)]

            # ------- conv1 + binarize -> A1 (2 images via PE row tiling) ------
            def conv1_pair(k):
                # images 2k (partitions 0-8) and 2k+1 (partitions 64-72)
                pq = k % 2
                nc.sync.dma_start(IC[pq][0:9, :], dX[2 * k])
                nc.sync.dma_start(IC[pq][64:73, :], dX[2 * k + 1])
                bufs = [A1[(2 * k) % 4], A1[(2 * k + 1) % 4]]
                for r in range(8):
                    for g in range(2):
                        ps = pp.tile([64, 512], F32, tag="mm",
                                     name=f"ps_c1_{k}_{r}_{g}")
                        nc.tensor.matmul(
                            ps[:], cW1T[64 * g:64 * g + 9, :],
                            IC[pq][64 * g:64 * g + 9, r * 512:(r + 1) * 512],
                            start=True, stop=True)
                        a1vv = bufs[g][:, 0:S1].rearrange(
                            "p (a b) -> p a b", b=P1)
                        nc.vector.tensor_scalar(
                            a1vv[0:64, r * 8 + 1: r * 8 + 9, 1:65],
                            ps[:].rearrange("q (a b) -> q a b", b=64),
                            cT1[:], None, OP.is_gt)
                for g in range(2):
                    a1t = bufs[g]
                    # copy2 at partitions 64..127: copy2[q] = copy1[q + P1]
                    nc.sync.dma_start(a1t[64:128, 0:S1 - P1], a1t[0:64, P1:S1])
                    # shifted blocks for DoubleRow pair dim (both halves)
                    nc.sync.dma_start(a1t[:, BO1:BO1 + 4360], a1t[:, 1:4361])
                    nc.sync.dma_start(a1t[:, 2 * BO1:2 * BO1 + 4232],
                                      a1t[:, 130:4362])

            # ---------------- layer bodies ------------------------------------
            def l2_block(i):
                # out(q=y*66+x) needs img[q + ky*66 + kx]; copy1(lo)=img[q],
                # copy2(hi)=img[q+66] => partition half packs ky,ky+1;
                # j blocks: blk1[q]=img[q+1], blk2[q]=img[q+130].
                p = i % 2
                a1f = A1[i % 4][:, :]
                a1lo = A1[i % 4][0:64, :]
                a2vv = A2[p][:, 0:S1].rearrange("p (a b) -> p a b", b=P1)
                for ci, (y0, nr) in enumerate(L2CH):
                    q0 = y0 * P1
                    n = nr * P1
                    ps = pp.tile([128, 512], F32, tag="mm", name=f"ps_l2_{i}_{ci}")
                    # taps (0,0),(1,0) | (0,1),(1,1)
                    nc.tensor.matmul(
                        ps[:, 0:n], w2d1,
                        _ap(a1f, q0, [[BO1, 2], [1, n]]),
                        start=True, stop=False, perf_mode=DR)
                    # taps (0,2),(1,2) | (2,0),zero
                    nc.tensor.matmul(
                        ps[:, 0:n], w2d2,
                        _ap(a1f, q0 + 2, [[2 * BO1, 2], [1, n]]),
                        start=False, stop=False, perf_mode=DR)
                    # taps (2,1) | (2,2)  (copy1 partitions, +2 rows + 1 base)
                    nc.tensor.matmul(
                        ps[:, 0:n], w2d3,
                        _ap(a1lo, q0 + 2 * P1 + 1, [[BO1, 2], [1, n]]),
                        start=False, stop=True, perf_mode=DR)
                    nc.vector.tensor_scalar(
                        a2vv[:, y0 + 1:y0 + 1 + nr, 1:65],
                        ps[:, 0:n].rearrange("q (a b) -> q a b", b=P1)
                        [:, :, 0:64],
                        cT2[:], None, OP.is_gt)
                # shifted blocks for L3's DoubleRow pair dim: +66, +1
                nc.sync.dma_start(A2[p][:, BO1:BO1 + 4292], A2[p][:, 66:4358])
                nc.sync.dma_start(A2[p][:, 2 * BO1:2 * BO1 + 4352],
                                  A2[p][:, 1:4353])

            def l3_block(i):
                # stride 2: out(y,x) reads A2[2y+ky, 2x+kx]; DoubleRow pairs
                # {(0,kx),(1,kx)} via blk1 (+66) and {(2,0),(2,1)} via blk2
                # (+1); solo (2,2) plain.
                p = i % 2
                a2f = A2[p][:, :]
                for r in range(2):
                    y0 = r * 16
                    base = (2 * y0) * P1
                    ps = pp.tile([128, 512], F32, tag="mm", name=f"ps_l3_{i}_{r}")
                    psv = ps[:].rearrange("q (a b) -> q a b", b=32)
                    for kx in range(3):
                        nc.tensor.matmul(
                            psv,
                            cW3D[:, kx * 256:(kx + 1) * 256]
                            .rearrange("p (j m) -> p j m", j=2),
                            _ap(a2f, base + kx,
                                [[BO1, 2], [2 * P1, 16], [2, 32]]),
                            start=(kx == 0), stop=False, perf_mode=DR)
                    nc.tensor.matmul(
                        psv, cW3D3[:].rearrange("p (j m) -> p j m", j=2),
                        _ap(a2f, base + 2 * P1,
                            [[2 * BO1, 2], [2 * P1, 16], [2, 32]]),
                        start=False, stop=False, perf_mode=DR)
                    nc.tensor.matmul(
                        psv, cW3S[:],
                        _ap(a2f, base + 2 * P1 + 2, [[2 * P1, 16], [2, 32]]),
                        start=False, stop=True)
                    nc.vector.tensor_scalar(
                        a3v(p)[:, y0 + 1:y0 + 17, 1:33], psv,
                        cT3[:], None, OP.is_gt)
                # shifted blocks for L4's DoubleRow pair dim
                nc.sync.dma_start(A3[p][:, BO3:BO3 + 1160], A3[p][:, 1:1161])
                nc.sync.dma_start(A3[p][:, 2 * BO3:2 * BO3 + 1128],
                                  A3[p][:, 32:1160])

            def l4_block(i):
                p = i % 2
                a3f = A3[p][:, :]
                a4av = A4[p][:, 0:NPOS].rearrange("p (a b) -> p a b", b=32)
                a4bv = A4[p][0:64, NPOS:2 * NPOS].rearrange(
                    "p (a b) -> p a b", b=32)
                for ci, (y0, nr) in enumerate(L4CH):
                    q0 = y0 * P3
                    n = nr * P3
                    psa = pp.tile([128, 512], F32, tag="mb", bufs=2,
                                  name=f"ps_l4a_{i}_{ci}")
                    psb = pp.tile([64, 512], F32, tag="mb2", bufs=2,
                                  name=f"ps_l4b_{i}_{ci}")
                    for mb, (psx, wd, ws, mw) in enumerate(
                            [(0, cW4DA, cW4SA, 128), (1, cW4DB, cW4SB, 64)]):
                        psx = psa if mb == 0 else psb
                        wd = cW4DA if mb == 0 else cW4DB
                        ws = cW4SA if mb == 0 else cW4SB
                        mw = 128 if mb == 0 else 64
                        for pi, (oa, js) in enumerate(L4P):
                            nc.tensor.matmul(
                                psx[:, 0:n],
                                wd[:, pi * 2 * mw:(pi + 1) * 2 * mw]
                                .rearrange("p (j m) -> p j m", j=2),
                                _ap(a3f, q0 + oa, [[js, 2], [1, n]]),
                                start=(pi == 0), stop=False, perf_mode=DR)
                        nc.tensor.matmul(
                            psx[:, 0:n], ws[:, 0:mw],
                            A3[p][:, q0 + 2 * P3 + 2: q0 + 2 * P3 + 2 + n],
                            start=False, stop=True)
                    nc.vector.tensor_scalar(
                        a4av[:, y0:y0 + nr, 0:32],
                        psa[:, 0:n].rearrange("q (a b) -> q a b", b=P3)
                        [:, :, 0:32],
                        cT4a[:], None, OP.is_gt)
                    nc.vector.tensor_scalar(
                        a4bv[:, y0:y0 + nr, 0:32],
                        psb[:, 0:n].rearrange("q (a b) -> q a b", b=P3)
                        [:, :, 0:32],
                        cT4b[:], None, OP.is_gt)

            def l5_block(i):
                # 1x1 conv, K=192 packed as DoubleRow j over A4's two blocks
                p = i % 2
                a4f = A4[p][:, :]
                for c in range(2):
                    psa = pp.tile([128, 512], F32, tag="mb", bufs=2,
                                  name=f"ps_l5a_{i}_{c}")
                    psb = pp.tile([64, 512], F32, tag="mb2", bufs=2,
                                  name=f"ps_l5b_{i}_{c}")
                    rhs = _ap(a4f, c * 512, [[NPOS, 2], [1, 512]])
                    nc.tensor.matmul(
                        psa[:], cW5DA[:].rearrange("p (j m) -> p j m", j=2),
                        rhs, start=True, stop=True, perf_mode=DR)
                    nc.tensor.matmul(
                        psb[:], cW5DB[:].rearrange("p (j m) -> p j m", j=2),
                        rhs, start=True, stop=True, perf_mode=DR)
                    # h5 = relu(a5*conv + b5); GAP partial sums via accum_out
                    scra = wp.tile([128, 512], F32, tag="scr_a",
                                   name=f"scra_{i}_{c}")
                    scrb = wp.tile([64, 512], F32, tag="scr_b",
                                   name=f"scrb_{i}_{c}")
                    nc.scalar.activation(
                        scra[:], psa[:], ACT.Relu, bias=cB5a[:], scale=cA5a[:],
                        accum_out=MACCa[:, 2 * i + c: 2 * i + c + 1])
                    nc.scalar.activation(
                        scrb[:], psb[:], ACT.Relu, bias=cB5b[:], scale=cA5b[:],
                        accum_out=MACCb[:, 2 * i + c: 2 * i + c + 1])

            # ---------------- main pipeline -----------------------------------
            for _rep in range(reps):
                conv1_pair(0)
                for i in range(B):
                    if i % 2 == 0 and i // 2 + 1 < B // 2:
                        conv1_pair(i // 2 + 1)
                    l2_block(i)
                    l3_block(i)
                    l4_block(i)
                    l5_block(i)

            # ---------------- GAP/FC/softmax tail -----------------------------
            nc.vector.tensor_reduce(
                Msum[:, 0:B], MACCa[:].rearrange("p (i c) -> p i c", c=2),
                axis=AX.X, op=OP.add)
            nc.vector.tensor_reduce(
                MsumB[0:64, 0:B], MACCb[:].rearrange("p (i c) -> p i c", c=2),
                axis=AX.X, op=OP.add)

            psf = pp.tile([16, 12], F32, tag="fc", bufs=1, name="ps_fc")
            nc.tensor.matmul(psf[:], Msum[:, 0:B], cWTa[:],
                             start=True, stop=False)
            nc.tensor.matmul(psf[:], MsumB[:, 0:B], cWTb[:],
                             start=False, stop=True)

            negmax = cp.tile([16, 1], F32, tag="negmax", name="negmax")
            esum = cp.tile([16, 1], F32, tag="esum", name="esum")
            rsum = cp.tile([16, 1], F32, tag="rsum", name="rsum")
            etile = cp.tile([16, 12], F32, tag="etile", name="etile")
            yout = cp.tile([16, 12], F32, tag="yout", name="yout")

            nc.vector.tensor_reduce(negmax[:], psf[:], axis=AX.X, op=OP.max,
                                    negate=True)
            nc.scalar.activation(etile[:], psf[:], ACT.Exp, bias=negmax[:],
                                 scale=1.0, accum_out=esum[:])
            nc.vector.reciprocal(rsum[:], esum[:])
            nc.vector.tensor_scalar(yout[:], etile[:], rsum[:], None, OP.mult)
            nc.sync.dma_start(dY[:], yout[:])

    nc.compile()
    _CACHE[key] = nc
    return _CACHE


def _host_prep(inputs):
    """Fold BN into thresholds/affines; sign-binarize weights; build per-core
    input maps."""
    f32 = np.float32
    fp8 = mybir.dt.np(FP8)

    x = np.asarray(inputs["x"], f32)

    def inv(l):
        return (np.asarray(inputs[f"bn{l}_g"], f32)
                / np.sqrt(np.asarray(inputs[f"bn{l}_v"], f32) + np.float32(EPS)))

    invs = {l: inv(l) for l in (1, 2, 3, 4, 5)}
    for l in (1, 2, 3, 4):
        assert (invs[l] > 0).all(), f"bn{l} scale not positive"

    def thr(l):
        return (np.asarray(inputs[f"bn{l}_m"], f32)
                - np.asarray(inputs[f"bn{l}_b"], f32) / invs[l])

    t1 = (thr(1) - np.asarray(inputs["conv1_b"], f32)).reshape(64, 1)
    t2 = thr(2).reshape(128, 1)
    t3 = thr(3).reshape(128, 1)
    t4 = thr(4)
    a5 = invs[5]
    b5 = (np.asarray(inputs["bn5_b"], f32)
          - np.asarray(inputs["bn5_m"], f32) * invs[5])

    # conv1 weights -> lhsT [tap, cout], replicated at partition rows 0/64
    w1 = np.asarray(inputs["conv1_w"], f32)           # [64,1,3,3]
    w1t = np.ascontiguousarray(w1[:, 0].reshape(64, 9).T)  # [9, 64]
    w1t4 = np.zeros((128, 64), f32)
    w1t4[0:9] = w1t
    w1t4[64:73] = w1t

    sw2 = np.sign(np.asarray(inputs["w2"], f32))       # [128,64,3,3]
    sw3 = np.sign(np.asarray(inputs["w3"], f32))       # [128,128,3,3]
    sw4 = np.sign(np.asarray(inputs["w4"], f32))       # [192,128,3,3]
    sw5 = np.sign(np.asarray(inputs["w5"], f32))       # [192,192,1,1]

    # L2 DoubleRow packs: partitions = [ci(64) x ky-half], j = second tap dim
    # MM1: j -> kx in {0,1} over ky-halves {0,1}
    w2d1 = np.zeros((128, 2, 128), f32)
    for h in range(2):
        for j in range(2):
            w2d1[64 * h:64 * (h + 1), j] = sw2[:, :, h, j].T
    # MM2: j0 -> (ky=h, kx=2); j1 -> (2,0) on lo half, zero on hi half
    w2d2 = np.zeros((128, 2, 128), f32)
    for h in range(2):
        w2d2[64 * h:64 * (h + 1), 0] = sw2[:, :, h, 2].T
    w2d2[0:64, 1] = sw2[:, :, 2, 0].T
    # MM3 (lo partitions only): j0 -> (2,1); j1 -> (2,2)
    w2d3 = np.zeros((64, 2, 128), f32)
    w2d3[:, 0] = sw2[:, :, 2, 1].T
    w2d3[:, 1] = sw2[:, :, 2, 2].T

    # L3 DR packs: w3d[kx]: j=ky in {0,1}; w3d3: j=kx in {0,1} at ky=2
    w3d = np.zeros((128, 3, 2, 128), f32)
    for kx in range(3):
        for j in range(2):
            w3d[:, kx, j] = sw3[:, :, j, kx].T
    w3d3 = np.zeros((128, 2, 128), f32)
    for j in range(2):
        w3d3[:, j] = sw3[:, :, 2, j].T
    w3s = np.ascontiguousarray(sw3[:, :, 2, 2].T)

    # L4 DoubleRow pairs + solo (2,2)
    L4P = [((0, 0), (0, 1)), ((0, 2), (1, 0)), ((1, 1), (1, 2)),
           ((2, 0), (2, 1))]
    w4da = np.zeros((128, 4, 2, 128), f32)
    w4db = np.zeros((128, 4, 2, 64), f32)
    for pi, (ta, tb) in enumerate(L4P):
        for j, (ky, kx) in enumerate((ta, tb)):
            w4da[:, pi, j] = sw4[:128, :, ky, kx].T
            w4db[:, pi, j] = sw4[128:, :, ky, kx].T
    w4sa = np.ascontiguousarray(sw4[:128, :, 2, 2].T)   # [128, 128]
    w4sb = np.ascontiguousarray(sw4[128:, :, 2, 2].T)   # [128, 64]

    # L5 DR packs: j0 = channels 0-127; j1 = channels 128-191 (partitions
    # 0-63, zeros above)
    w5 = sw5[:, :, 0, 0]                               # [co=192, ci=192]
    w5da = np.zeros((128, 2, 128), f32)
    w5da[:, 0] = w5[:128, :128].T
    w5da[0:64, 1] = w5[:128, 128:].T
    w5db = np.zeros((128, 2, 64), f32)
    w5db[:, 0] = w5[128:, :128].T
    w5db[0:64, 1] = w5[128:, 128:].T

    fc_w = np.asarray(inputs["fc_w"], f32)
    c6w = np.asarray(inputs["conv6_w"], f32)[:, :, 0, 0]   # [12, 192]
    Wp = (fc_w @ c6w) / np.float32(NPOS)               # [12, 192]
    cvec = fc_w @ np.asarray(inputs["conv6_b"], f32) + np.asarray(
        inputs["fc_b"], f32)                           # [12]
    wta = np.ascontiguousarray(Wp[:, :128].T)          # [128, 12]
    wtb = np.zeros((65, 12), f32)
    wtb[:64] = Wp[:, 128:].T
    wtb[64] = cvec

    shared = {
        "w1t4": w1t4.astype(f32),
        "w2d1": w2d1.reshape(128, 256).astype(fp8),
        "w2d2": w2d2.reshape(128, 256).astype(fp8),
        "w2d3": w2d3.reshape(64, 256).astype(fp8),
        "w3d": w3d.reshape(128, 768).astype(fp8),
        "w3d3": w3d3.reshape(128, 256).astype(fp8),
        "w3s": w3s.astype(fp8),
        "w4da": w4da.reshape(128, 1024).astype(fp8),
        "w4sa": w4sa.astype(fp8),
        "w4db": w4db.reshape(128, 512).astype(fp8),
        "w4sb": w4sb.astype(fp8),
        "w5da": w5da.reshape(128, 256).astype(fp8),
        "w5db": w5db.reshape(128, 128).astype(fp8),
        "t1": t1.astype(f32), "t2": t2.astype(f32), "t3": t3.astype(f32),
        "t4a": t4[:128].reshape(128, 1).astype(f32),
        "t4b": t4[128:].reshape(64, 1).astype(f32),
        "a5a": a5[:128].reshape(128, 1).astype(f32),
        "a5b": a5[128:].reshape(64, 1).astype(f32),
        "b5a": b5[:128].reshape(128, 1).astype(f32),
        "b5b": b5[128:].reshape(64, 1).astype(f32),
        "wta": wta.astype(f32), "wtb": wtb.astype(f32),
    }
    # host im2col: cols[b, 3*ky+kx, y*64+x] = xpad[b, 2y+ky, 2x+kx]
    xpad = np.pad(x[:, 0], ((0, 0), (1, 1), (1, 1)))
    cols = np.stack([xpad[:, ky:ky + 127:2, kx:kx + 127:2]
                     for ky in range(3) for kx in range(3)],
                    axis=1).reshape(x.shape[0], 9, 4096)
    in_maps = []
    for c in range(N_CORES):
        m = dict(shared)
        m["x"] = np.ascontiguousarray(cols[c * B:(c + 1) * B])
        in_maps.append(m)
    return in_maps


def kernel(**inputs):
    cache = _build()
    in_maps = _host_prep(inputs)
    res = run_bass_kernel_spmd(cache["nc1"], in_maps,
                               core_ids=list(range(N_CORES)))
    _CACHE["last_results"] = res
    return np.concatenate([res.results[c]["y"] for c in range(N_CORES)],
                          axis=0)


# ---------------------------------------------------------------------------
# numpy golden model of the device algorithm (for fast validation in test.py)
# ---------------------------------------------------------------------------
def golden(inputs):
    f32 = np.float32
    in_maps = _host_prep(inputs)
    outs = []
    for m in in_maps:
        cols = np.asarray(m["x"], f32)  # [B, 9, 4096] host im2col
        t1 = m["t1"][:, 0]
        w1t = m["w1t4"][0:9]
        c1 = np.einsum("btn,tc->bcn", cols, w1t).reshape(-1, 64, 64, 64)
        a1 = (c1 > t1[None, :, None, None]).astype(f32)

        def bconv(a, wt, stride):
            # a: [B,C,H,W] binary; wt[ci, t, co] for t = 3*ky+kx
            Bn, C, H, W = a.shape
            ap = np.pad(a, ((0, 0), (0, 0), (1, 1), (1, 1)))
            Ho, Wo = H // stride, W // stride
            out = np.zeros((Bn, wt.shape[2], Ho, Wo), f32)
            for t in range(9):
                ky, kx = t // 3, t % 3
                sl = ap[:, :, ky:ky + H:stride, kx:kx + W:stride][
                    :, :, :Ho, :Wo]
                out += np.einsum("bcyx,cd->bdyx", sl, wt[:, t])
            return out

        # reconstruct w2 [ci, t, co] from the DR packs
        w2d1 = np.asarray(m["w2d1"], f32).reshape(128, 2, 128)
        w2d2 = np.asarray(m["w2d2"], f32).reshape(128, 2, 128)
        w2d3 = np.asarray(m["w2d3"], f32).reshape(64, 2, 128)
        w2 = np.zeros((64, 9, 128), f32)
        for h in range(2):
            for j in range(2):
                w2[:, 3 * h + j] = w2d1[64 * h:64 * h + 64, j]
            w2[:, 3 * h + 2] = w2d2[64 * h:64 * h + 64, 0]
        w2[:, 6] = w2d2[0:64, 1]
        w2[:, 7] = w2d3[:, 0]
        w2[:, 8] = w2d3[:, 1]
        c2 = bconv(a1, w2, 1)
        a2 = (c2 > m["t2"].reshape(1, 128, 1, 1)).astype(f32)

        w3dg = np.asarray(m["w3d"], f32).reshape(128, 3, 2, 128)
        w3d3g = np.asarray(m["w3d3"], f32).reshape(128, 2, 128)
        w3 = np.zeros((128, 9, 128), f32)
        for kx in range(3):
            for j in range(2):
                w3[:, 3 * j + kx] = w3dg[:, kx, j]
        w3[:, 6] = w3d3g[:, 0]
        w3[:, 7] = w3d3g[:, 1]
        w3[:, 8] = np.asarray(m["w3s"], f32)
        c3 = bconv(a2, w3, 2)
        a3 = (c3 > m["t3"].reshape(1, 128, 1, 1)).astype(f32)

        # reconstruct w4 [ci, t, co] from DR packs + solo
        L4P = [((0, 0), (0, 1)), ((0, 2), (1, 0)), ((1, 1), (1, 2)),
               ((2, 0), (2, 1))]
        w4da = np.asarray(m["w4da"], f32).reshape(128, 4, 2, 128)
        w4db = np.asarray(m["w4db"], f32).reshape(128, 4, 2, 64)
        w4 = np.zeros((128, 9, 192), f32)
        for pi, (ta, tb) in enumerate(L4P):
            for j, (ky, kx) in enumerate((ta, tb)):
                w4[:, 3 * ky + kx, :128] = w4da[:, pi, j]
                w4[:, 3 * ky + kx, 128:] = w4db[:, pi, j]
        w4[:, 8, :128] = np.asarray(m["w4sa"], f32)
        w4[:, 8, 128:] = np.asarray(m["w4sb"], f32)
        c4 = bconv(a3, w4, 1)
        a4 = np.concatenate([
            (c4[:, :128] > m["t4a"].reshape(1, 128, 1, 1)).astype(f32),
            (c4[:, 128:] > m["t4b"].reshape(1, 64, 1, 1)).astype(f32)], axis=1)

        w5dag = np.asarray(m["w5da"], f32).reshape(128, 2, 128)
        w5dbg = np.asarray(m["w5db"], f32).reshape(128, 2, 64)
        w5 = np.zeros((192, 192), f32)  # [ci, co]
        w5[:128, :128] = w5dag[:, 0]
        w5[128:, :128] = w5dag[0:64, 1]
        w5[:128, 128:] = w5dbg[:, 0]
        w5[128:, 128:] = w5dbg[0:64, 1]
        c5 = np.einsum("bcyx,cd->bdyx", a4, w5)
        a5v = np.concatenate([m["a5a"], m["a5b"]], axis=0).reshape(1, 192, 1, 1)
        b5v = np.concatenate([m["b5a"], m["b5b"]], axis=0).reshape(1, 192, 1, 1)
        h5 = np.maximum(a5v * c5 + b5v, 0.0)
        sums = h5.sum(axis=(2, 3))  # [B, 192]
        WT = np.concatenate([m["wta"], m["wtb"][:64]], axis=0)  # [192, 12]
        logits = sums @ WT + m["wtb"][64][None, :]
        z = logits - logits.max(axis=1, keepdims=True)
        e = np.exp(z)
        outs.append(e / e.sum(axis=1, keepdims=True))
    return np.concatenate(outs, axis=0)


# revision 69
# speedup vs baseline: 5.7464x; 1.4829x over previous
"""Trainium2 Bass kernel for BinarizedInputNetwork.

Contract: kernel(**inputs) takes the FULL unsharded inputs (batch 128) and
returns the FULL [128, 12] float32 softmax output. Internally shards the
batch across 8 NeuronCores (16 images each), runs one SPMD Bass program.

Network (per image, input [1,128,128]):
  conv1 3x3 s2 p1 (1->64)  + BN + ReLU -> sign       => binary acts {0,1}
  conv2 3x3 s1 p1 (64->128, sign wts)  + BN + ReLU -> sign
  conv3 3x3 s2 p1 (128->128, sign wts) + BN + ReLU -> sign
  conv4 3x3 s1 p1 (128->192, sign wts) + BN + ReLU -> sign
  conv5 1x1 s1 p0 (192->192, sign wts) + BN + ReLU
  conv6 1x1 (192->12) + b ; GAP ; FC 12x12 + b ; softmax

Device mapping:
  - Convs are shifted matmuls: activations live in SBUF as [C, Hp*Wp]
    (channel on partition, zero-padded spatial); each 3x3 tap is an
    accumulating matmul (K=Cin, M=Cout, N=outputs chunk).
  - sign(relu(bn(x))) == (x > t_c) per-channel (bn scale positive): one
    VectorE tensor_scalar is_gt PSUM->SBUF. Acts {0,1} and weights {-1,0,1}
    are exact in fp8e4m3, so layers 2-5 are exact integer arithmetic.
  - fp8 + DoubleRow packs 2 taps per matmul via the [Ki, 2, N] pair dim
    (arbitrary j-step in SBUF). L2 additionally packs 2 more taps on the
    partition dim via a second input copy shifted one row (K_eff=256).
  - conv6+GAP+FC folded: GAP via ScalarE activation accum_out; logits =
    (fc_w@conv6_w/1024) @ sums + (fc_w@conv6_b + fc_b) via one tiny matmul
    with a ones-row; softmax on device.
"""

import sys

sys.path.insert(0, "/opt/trn_rl_repo")

import numpy as np

import concourse.ap as apm
import concourse.bass as bass
import concourse.mybir as mybir
import concourse.bacc as bacc
import concourse.tile as tile
from concourse.bass_utils import run_bass_kernel_spmd

F32 = mybir.dt.float32
FP8 = mybir.dt.float8e4
AX = mybir.AxisListType
OP = mybir.AluOpType
ACT = mybir.ActivationFunctionType
DR = mybir.MatmulPerfMode.DoubleRow

N_CORES = 8
B = 16  # images per core

EPS = 1e-5

# geometry
H1, W1 = 64, 64          # conv1 output spatial
P1 = W1 + 2              # padded width/height for A1/A2 (66)
S1 = P1 * P1             # 4356
S1e = S1 + 8             # guard tail
BO1 = 4368               # A1 shifted-copy block offset (16-aligned)
H3, W3 = 32, 32          # conv3 output spatial
P3 = W3 + 2              # 34
# A2/A3 use 16-aligned row pitches so row-pair DoubleRow taps need no
# shifted copies (j-step = pitch); only the +1-column pair needs one block.
P2A = 80                 # A2 row pitch (66 rows x 80)
S2A = 66 * P2A           # 5280
P3A = 48                 # A3 row pitch (34 rows x 48)
S3A = P3 * P3A           # 1632
NPOS = H3 * W3           # 1024 valid positions for L5/GAP

_CACHE = {}


def _ap(base2d, off, dims):
    """Custom AP over an SBUF tile slice: base partition dim + free dims
    (supports overlapping patterns rearrange can't express)."""
    return apm.AP(tensor=base2d.tensor, offset=base2d.offset + off,
                  ap=[list(base2d.ap[0])] + [list(d) for d in dims])


def _build(reps=1):
    """Trace + compile the Bass program (cached). reps>1 replicates the whole
    pipeline on-device (for timing via wall-clock differencing)."""
    key = f"nc{reps}"
    if key in _CACHE:
        return _CACHE

    nc = bacc.Bacc("TRN2", target_bir_lowering=False, debug=False,
                   num_devices=N_CORES)

    # ---- DRAM I/O ----
    # host-side im2col of the 1-channel input: [B, 9, 64*64]
    dX = nc.dram_tensor("x", [B, 9, 4096], F32, kind="ExternalInput").ap()
    # conv1 weights at partition rows 0-8 and 64-72 (2-way row tiling)
    dW1T = nc.dram_tensor("w1t4", [128, 64], F32, kind="ExternalInput").ap()
    dW2D1 = nc.dram_tensor("w2d1", [128, 256], FP8, kind="ExternalInput").ap()
    dW2D2 = nc.dram_tensor("w2d2", [128, 256], FP8, kind="ExternalInput").ap()
    dW2D3 = nc.dram_tensor("w2d3", [64, 256], FP8, kind="ExternalInput").ap()
    dW3D = nc.dram_tensor("w3d", [128, 768], FP8, kind="ExternalInput").ap()
    dW3D3 = nc.dram_tensor("w3d3", [128, 256], FP8, kind="ExternalInput").ap()
    dW3S = nc.dram_tensor("w3s", [128, 128], FP8, kind="ExternalInput").ap()
    dW4DA = nc.dram_tensor("w4da", [128, 768], FP8, kind="ExternalInput").ap()
    dW4D3A = nc.dram_tensor("w4d3a", [128, 256], FP8, kind="ExternalInput").ap()
    dW4SA = nc.dram_tensor("w4sa", [128, 128], FP8, kind="ExternalInput").ap()
    dW4DB = nc.dram_tensor("w4db", [128, 384], FP8, kind="ExternalInput").ap()
    dW4D3B = nc.dram_tensor("w4d3b", [128, 128], FP8, kind="ExternalInput").ap()
    dW4SB = nc.dram_tensor("w4sb", [128, 64], FP8, kind="ExternalInput").ap()
    dW5DA = nc.dram_tensor("w5da", [128, 256], FP8, kind="ExternalInput").ap()
    dW5DB = nc.dram_tensor("w5db", [128, 128], FP8, kind="ExternalInput").ap()
    dT1 = nc.dram_tensor("t1", [64, 1], F32, kind="ExternalInput").ap()
    dT2 = nc.dram_tensor("t2", [128, 1], F32, kind="ExternalInput").ap()
    dT3 = nc.dram_tensor("t3", [128, 1], F32, kind="ExternalInput").ap()
    dT4a = nc.dram_tensor("t4a", [128, 1], F32, kind="ExternalInput").ap()
    dT4b = nc.dram_tensor("t4b", [64, 1], F32, kind="ExternalInput").ap()
    dA5a = nc.dram_tensor("a5a", [128, 1], F32, kind="ExternalInput").ap()
    dA5b = nc.dram_tensor("a5b", [64, 1], F32, kind="ExternalInput").ap()
    dB5a = nc.dram_tensor("b5a", [128, 1], F32, kind="ExternalInput").ap()
    dB5b = nc.dram_tensor("b5b", [64, 1], F32, kind="ExternalInput").ap()
    dWTa = nc.dram_tensor("wta", [128, 12], F32, kind="ExternalInput").ap()
    dWTb = nc.dram_tensor("wtb", [65, 12], F32, kind="ExternalInput").ap()
    dY = nc.dram_tensor("y", [B, 12], F32, kind="ExternalOutput").ap()

    with tile.TileContext(nc) as tc:
        with tc.tile_pool(name="const", bufs=1) as cp, \
             tc.tile_pool(name="work", bufs=2) as wp, \
             tc.tile_pool(name="psum", bufs=3, space="PSUM") as pp:

            def ctile(name, shape, dtype):
                return cp.tile(shape, dtype, tag=name, name=name)

            # ---- persistent weight/param tiles ----
            cW1T = ctile("cW1T", [128, 64], F32)
            cW2D1 = ctile("cW2D1", [128, 256], FP8)
            cW2D2 = ctile("cW2D2", [128, 256], FP8)
            cW2D3 = ctile("cW2D3", [64, 256], FP8)
            cW3D = ctile("cW3D", [128, 768], FP8)
            cW3D3 = ctile("cW3D3", [128, 256], FP8)
            cW3S = ctile("cW3S", [128, 128], FP8)
            cW4DA = ctile("cW4DA", [128, 768], FP8)
            cW4D3A = ctile("cW4D3A", [128, 256], FP8)
            cW4SA = ctile("cW4SA", [128, 128], FP8)
            cW4DB = ctile("cW4DB", [128, 384], FP8)
            cW4D3B = ctile("cW4D3B", [128, 128], FP8)
            cW4SB = ctile("cW4SB", [128, 64], FP8)
            cW5DA = ctile("cW5DA", [128, 256], FP8)
            cW5DB = ctile("cW5DB", [128, 128], FP8)
            cT1 = ctile("cT1", [64, 1], F32)
            cT2 = ctile("cT2", [128, 1], F32)
            cT3 = ctile("cT3", [128, 1], F32)
            cT4a = ctile("cT4a", [128, 1], F32)
            cT4b = ctile("cT4b", [64, 1], F32)
            cA5a = ctile("cA5a", [128, 1], F32)
            cA5b = ctile("cA5b", [64, 1], F32)
            cB5a = ctile("cB5a", [128, 1], F32)
            cB5b = ctile("cB5b", [64, 1], F32)
            cWTa = ctile("cWTa", [128, 12], F32)
            cWTb = ctile("cWTb", [65, 12], F32)

            for t_, d_ in [(cW1T, dW1T), (cW2D1, dW2D1), (cW2D2, dW2D2),
                           (cW3D, dW3D), (cW3D3, dW3D3), (cW3S, dW3S),
                           (cW4DA, dW4DA), (cW4D3A, dW4D3A), (cW4SA, dW4SA),
                           (cW4DB, dW4DB), (cW4D3B, dW4D3B),
                           (cW4SB, dW4SB), (cW5DA, dW5DA),
                           (cW5DB, dW5DB), (cT1, dT1), (cT2, dT2), (cT3, dT3),
                           (cT4a, dT4a), (cT4b, dT4b), (cA5a, dA5a),
                           (cA5b, dA5b), (cB5a, dB5a), (cB5b, dB5b),
                           (cWTa, dWTa), (cWTb, dWTb)]:
                nc.sync.dma_start(t_[:], d_[:])
            nc.sync.dma_start(cW2D3[:], dW2D3[:])

            # ---- persistent activation buffers (double-buffered by parity) ---
            # IC: image pair (g=0 at partitions 0-8, g=1 at 64-72)
            IC = [ctile(f"IC{p}", [128, 4096], F32) for p in range(2)]
            # A1/A2/A3 carry shifted-copy blocks at 16-aligned offsets for
            # DoubleRow's pair-dim step. A1: {+0, +1, +130}; A2: {+0, +66, +1};
            # A3: {+0, +1, +32}.
            A1 = [ctile(f"A1_{p}", [128, 3 * BO1], FP8) for p in range(4)]
            A2 = [ctile(f"A2_{p}", [128, 2 * S2A], FP8) for p in range(2)]
            A3 = [ctile(f"A3_{p}", [128, 2 * S3A], FP8) for p in range(2)]
            # A4: block0 = channels 0-127; block1 (offset NPOS) = channels
            # 128-191 on partitions 0-63, zeros above
            A4 = [ctile(f"A4_{p}", [128, 2 * NPOS], FP8) for p in range(2)]
            MACCa = ctile("MACCa", [128, 2 * B], F32)
            MACCb = ctile("MACCb", [64, 2 * B], F32)
            Msum = ctile("Msum", [128, B], F32)
            MsumB = ctile("MsumB", [65, B], F32)

            # zero padding once; interiors are rewritten every image
            for p in range(4):
                nc.gpsimd.memset(A1[p][:], 0.0)
            for p in range(2):
                nc.gpsimd.memset(A2[p][:], 0.0)
                nc.gpsimd.memset(A3[p][:], 0.0)
                nc.gpsimd.memset(A4[p][64:128, NPOS:2 * NPOS], 0.0)
            nc.vector.memset(MsumB[64:65, :], 1.0)

            def a3v(p):
                return A3[p][:, 0:S3A].rearrange("p (a b) -> p a b", b=P3A)

            w2d1 = cW2D1[:].rearrange("p (j m) -> p j m", j=2)
            w2d2 = cW2D2[:].rearrange("p (j m) -> p j m", j=2)
            w2d3 = cW2D3[:].rearrange("p (j m) -> p j m", j=2)

            # L2 row-aligned chunks (valid-only strided epilogue)
            L2CH = [(r * 7, 7) for r in range(9)] + [(63, 1)]
            # L4 row-aligned chunks (A4 is unpadded)
            L4CH = [(0, 15), (15, 15), (30, 2)]

            # ------- conv1 + binarize -> A1 (2 images via PE row tiling) ------
            def conv1_pair(k):
                # images 2k (partitions 0-8) and 2k+1 (partitions 64-72)
                pq = k % 2
                nc.sync.dma_start(IC[pq][0:9, :], dX[2 * k])
                nc.sync.dma_start(IC[pq][64:73, :], dX[2 * k + 1])
                bufs = [A1[(2 * k) % 4], A1[(2 * k + 1) % 4]]
                for r in range(8):
                    for g in range(2):
                        ps = pp.tile([64, 512], F32, tag="mm",
                                     name=f"ps_c1_{k}_{r}_{g}")
                        nc.tensor.matmul(
                            ps[:], cW1T[64 * g:64 * g + 9, :],
                            IC[pq][64 * g:64 * g + 9, r * 512:(r + 1) * 512],
                            start=True, stop=True)
                        a1vv = bufs[g][:, 0:S1].rearrange(
                            "p (a b) -> p a b", b=P1)
                        nc.vector.tensor_scalar(
                            a1vv[0:64, r * 8 + 1: r * 8 + 9, 1:65],
                            ps[:].rearrange("q (a b) -> q a b", b=64),
                            cT1[:], None, OP.is_gt)
                for g in range(2):
                    a1t = bufs[g]
                    # copy2 at partitions 64..127: copy2[q] = copy1[q + P1]
                    nc.sync.dma_start(a1t[64:128, 0:S1 - P1], a1t[0:64, P1:S1])
                    # shifted blocks for DoubleRow pair dim (both halves)
                    nc.sync.dma_start(a1t[:, BO1:BO1 + 4360], a1t[:, 1:4361])
                    nc.sync.dma_start(a1t[:, 2 * BO1:2 * BO1 + 4232],
                                      a1t[:, 130:4362])

            # ---------------- layer bodies ------------------------------------
            def l2_block(i):
                # out(q=y*66+x) needs img[q + ky*66 + kx]; copy1(lo)=img[q],
                # copy2(hi)=img[q+66] => partition half packs ky,ky+1;
                # j blocks: blk1[q]=img[q+1], blk2[q]=img[q+130].
                p = i % 2
                a1f = A1[i % 4][:, :]
                a1lo = A1[i % 4][0:64, :]
                a2vv = A2[p][:, 0:S2A].rearrange("p (a b) -> p a b", b=P2A)
                for ci, (y0, nr) in enumerate(L2CH):
                    q0 = y0 * P1
                    n = nr * P1
                    ps = pp.tile([128, 512], F32, tag="mm", name=f"ps_l2_{i}_{ci}")
                    # taps (0,0),(1,0) | (0,1),(1,1)
                    nc.tensor.matmul(
                        ps[:, 0:n], w2d1,
                        _ap(a1f, q0, [[BO1, 2], [1, n]]),
                        start=True, stop=False, perf_mode=DR)
                    # taps (0,2),(1,2) | (2,0),zero
                    nc.tensor.matmul(
                        ps[:, 0:n], w2d2,
                        _ap(a1f, q0 + 2, [[2 * BO1, 2], [1, n]]),
                        start=False, stop=False, perf_mode=DR)
                    # taps (2,1) | (2,2)  (copy1 partitions, +2 rows + 1 base)
                    nc.tensor.matmul(
                        ps[:, 0:n], w2d3,
                        _ap(a1lo, q0 + 2 * P1 + 1, [[BO1, 2], [1, n]]),
                        start=False, stop=True, perf_mode=DR)
                    nc.vector.tensor_scalar(
                        a2vv[:, y0 + 1:y0 + 1 + nr, 1:65],
                        ps[:, 0:n].rearrange("q (a b) -> q a b", b=P1)
                        [:, :, 0:64],
                        cT2[:], None, OP.is_gt)
                    # per-chunk +1-col shifted block: blk1[q] = A2[q+1]
                    # (outside regions are zero on both sides)
                    ql = (y0 + 1) * P2A + 1
                    qh = (y0 + nr) * P2A + 65
                    nc.sync.dma_start(
                        A2[p][:, S2A + ql - 1:S2A + qh - 1], A2[p][:, ql:qh])

            def l3_block(i):
                # stride 2: out(y,x) reads A2[2y+ky, 2x+kx]; row pairs
                # {(0,kx),(1,kx)} use j-step = pitch (80); {(2,0),(2,1)} via
                # blk1 (+1 col, j-step S2A); solo (2,2) plain.
                p = i % 2
                a2f = A2[p][:, :]
                for r in range(2):
                    y0 = r * 16
                    base = (2 * y0) * P2A
                    ps = pp.tile([128, 512], F32, tag="mm", name=f"ps_l3_{i}_{r}")
                    psv = ps[:].rearrange("q (a b) -> q a b", b=32)
                    for kx in range(3):
                        nc.tensor.matmul(
                            psv,
                            cW3D[:, kx * 256:(kx + 1) * 256]
                            .rearrange("p (j m) -> p j m", j=2),
                            _ap(a2f, base + kx,
                                [[P2A, 2], [2 * P2A, 16], [2, 32]]),
                            start=(kx == 0), stop=False, perf_mode=DR)
                    nc.tensor.matmul(
                        psv, cW3D3[:].rearrange("p (j m) -> p j m", j=2),
                        _ap(a2f, base + 2 * P2A,
                            [[S2A, 2], [2 * P2A, 16], [2, 32]]),
                        start=False, stop=False, perf_mode=DR)
                    nc.tensor.matmul(
                        psv, cW3S[:],
                        _ap(a2f, base + 2 * P2A + 2, [[2 * P2A, 16], [2, 32]]),
                        start=False, stop=True)
                    nc.vector.tensor_scalar(
                        a3v(p)[:, y0 + 1:y0 + 17, 1:33], psv,
                        cT3[:], None, OP.is_gt)
                    # per-chunk +1-col shifted block: blk1[q] = A3[q+1]
                    ql = (y0 + 1) * P3A + 1
                    qh = (y0 + 16) * P3A + 33
                    nc.sync.dma_start(
                        A3[p][:, S3A + ql - 1:S3A + qh - 1], A3[p][:, ql:qh])

            def l4_block(i):
                # row pairs {(0,kx),(1,kx)} use j-step = pitch (48);
                # {(2,0),(2,1)} via blk1 (+1 col, j-step S3A); solo (2,2).
                p = i % 2
                a3f = A3[p][:, :]
                a4av = A4[p][:, 0:NPOS].rearrange("p (a b) -> p a b", b=32)
                a4bv = A4[p][0:64, NPOS:2 * NPOS].rearrange(
                    "p (a b) -> p a b", b=32)
                for ci, (y0, nr) in enumerate(L4CH):
                    q0 = y0 * P3A
                    n = nr * P3
                    psa = pp.tile([128, 512], F32, tag="mb", bufs=2,
                                  name=f"ps_l4a_{i}_{ci}")
                    psb = pp.tile([64, 512], F32, tag="mb2", bufs=2,
                                  name=f"ps_l4b_{i}_{ci}")
                    for mb in range(2):
                        psx = psa if mb == 0 else psb
                        wd = cW4DA if mb == 0 else cW4DB
                        wd3 = cW4D3A if mb == 0 else cW4D3B
                        ws = cW4SA if mb == 0 else cW4SB
                        mw = 128 if mb == 0 else 64
                        for kx in range(3):
                            nc.tensor.matmul(
                                psx[:, 0:n],
                                wd[:, kx * 2 * mw:(kx + 1) * 2 * mw]
                                .rearrange("p (j m) -> p j m", j=2),
                                _ap(a3f, q0 + kx,
                                    [[P3A, 2], [P3A, nr], [1, P3]]),
                                start=(kx == 0), stop=False, perf_mode=DR)
                        nc.tensor.matmul(
                            psx[:, 0:n],
                            wd3[:, 0:2 * mw].rearrange("p (j m) -> p j m", j=2),
                            _ap(a3f, q0 + 2 * P3A,
                                [[S3A, 2], [P3A, nr], [1, P3]]),
                            start=False, stop=False, perf_mode=DR)
                        nc.tensor.matmul(
                            psx[:, 0:n], ws[:, 0:mw],
                            _ap(a3f, q0 + 2 * P3A + 2, [[P3A, nr], [1, P3]]),
                            start=False, stop=True)
                    nc.vector.tensor_scalar(
                        a4av[:, y0:y0 + nr, 0:32],
                        psa[:, 0:n].rearrange("q (a b) -> q a b", b=P3)
                        [:, :, 0:32],
                        cT4a[:], None, OP.is_gt)
                    nc.vector.tensor_scalar(
                        a4bv[:, y0:y0 + nr, 0:32],
                        psb[:, 0:n].rearrange("q (a b) -> q a b", b=P3)
                        [:, :, 0:32],
                        cT4b[:], None, OP.is_gt)

            def l5_block(i):
                # 1x1 conv, K=192 packed as DoubleRow j over A4's two blocks
                p = i % 2
                a4f = A4[p][:, :]
                for c in range(2):
                    psa = pp.tile([128, 512], F32, tag="mb", bufs=2,
                                  name=f"ps_l5a_{i}_{c}")
                    psb = pp.tile([64, 512], F32, tag="mb2", bufs=2,
                                  name=f"ps_l5b_{i}_{c}")
                    rhs = _ap(a4f, c * 512, [[NPOS, 2], [1, 512]])
                    nc.tensor.matmul(
                        psa[:], cW5DA[:].rearrange("p (j m) -> p j m", j=2),
                        rhs, start=True, stop=True, perf_mode=DR)
                    nc.tensor.matmul(
                        psb[:], cW5DB[:].rearrange("p (j m) -> p j m", j=2),
                        rhs, start=True, stop=True, perf_mode=DR)
                    # h5 = relu(a5*conv + b5); GAP partial sums via accum_out
                    scra = wp.tile([128, 512], F32, tag="scr_a",
                                   name=f"scra_{i}_{c}")
                    scrb = wp.tile([64, 512], F32, tag="scr_b",
                                   name=f"scrb_{i}_{c}")
                    nc.scalar.activation(
                        scra[:], psa[:], ACT.Relu, bias=cB5a[:], scale=cA5a[:],
                        accum_out=MACCa[:, 2 * i + c: 2 * i + c + 1])
                    nc.scalar.activation(
                        scrb[:], psb[:], ACT.Relu, bias=cB5b[:], scale=cA5b[:],
                        accum_out=MACCb[:, 2 * i + c: 2 * i + c + 1])

            # ---------------- main pipeline -----------------------------------
            for _rep in range(reps):
                conv1_pair(0)
                for i in range(B):
                    if i % 2 == 0 and i // 2 + 1 < B // 2:
                        conv1_pair(i // 2 + 1)
                    l2_block(i)
                    l3_block(i)
                    l4_block(i)
                    l5_block(i)

            # ---------------- GAP/FC/softmax tail -----------------------------
            nc.vector.tensor_reduce(
                Msum[:, 0:B], MACCa[:].rearrange("p (i c) -> p i c", c=2),
                axis=AX.X, op=OP.add)
            nc.vector.tensor_reduce(
                MsumB[0:64, 0:B], MACCb[:].rearrange("p (i c) -> p i c", c=2),
                axis=AX.X, op=OP.add)

            psf = pp.tile([16, 12], F32, tag="fc", bufs=1, name="ps_fc")
            nc.tensor.matmul(psf[:], Msum[:, 0:B], cWTa[:],
                             start=True, stop=False)
            nc.tensor.matmul(psf[:], MsumB[:, 0:B], cWTb[:],
                             start=False, stop=True)

            negmax = cp.tile([16, 1], F32, tag="negmax", name="negmax")
            esum = cp.tile([16, 1], F32, tag="esum", name="esum")
            rsum = cp.tile([16, 1], F32, tag="rsum", name="rsum")
            etile = cp.tile([16, 12], F32, tag="etile", name="etile")
            yout = cp.tile([16, 12], F32, tag="yout", name="yout")

            nc.vector.tensor_reduce(negmax[:], psf[:], axis=AX.X, op=OP.max,
                                    negate=True)
            nc.scalar.activation(etile[:], psf[:], ACT.Exp, bias=negmax[:],
                                 scale=1.0, accum_out=esum[:])
            nc.vector.reciprocal(rsum[:], esum[:])
            nc.vector.tensor_scalar(yout[:], etile[:], rsum[:], None, OP.mult)
            nc.sync.dma_start(dY[:], yout[:])

    nc.compile()
    _CACHE[key] = nc
    return _CACHE


def _host_prep(inputs):
    """Fold BN into thresholds/affines; sign-binarize weights; build per-core
    input maps."""
    f32 = np.float32
    fp8 = mybir.dt.np(FP8)

    x = np.asarray(inputs["x"], f32)

    def inv(l):
        return (np.asarray(inputs[f"bn{l}_g"], f32)
                / np.sqrt(np.asarray(inputs[f"bn{l}_v"], f32) + np.float32(EPS)))

    invs = {l: inv(l) for l in (1, 2, 3, 4, 5)}
    for l in (1, 2, 3, 4):
        assert (invs[l] > 0).all(), f"bn{l} scale not positive"

    def thr(l):
        return (np.asarray(inputs[f"bn{l}_m"], f32)
                - np.asarray(inputs[f"bn{l}_b"], f32) / invs[l])

    t1 = (thr(1) - np.asarray(inputs["conv1_b"], f32)).reshape(64, 1)
    t2 = thr(2).reshape(128, 1)
    t3 = thr(3).reshape(128, 1)
    t4 = thr(4)
    a5 = invs[5]
    b5 = (np.asarray(inputs["bn5_b"], f32)
          - np.asarray(inputs["bn5_m"], f32) * invs[5])

    # conv1 weights -> lhsT [tap, cout], replicated at partition rows 0/64
    w1 = np.asarray(inputs["conv1_w"], f32)           # [64,1,3,3]
    w1t = np.ascontiguousarray(w1[:, 0].reshape(64, 9).T)  # [9, 64]
    w1t4 = np.zeros((128, 64), f32)
    w1t4[0:9] = w1t
    w1t4[64:73] = w1t

    sw2 = np.sign(np.asarray(inputs["w2"], f32))       # [128,64,3,3]
    sw3 = np.sign(np.asarray(inputs["w3"], f32))       # [128,128,3,3]
    sw4 = np.sign(np.asarray(inputs["w4"], f32))       # [192,128,3,3]
    sw5 = np.sign(np.asarray(inputs["w5"], f32))       # [192,192,1,1]

    # L2 DoubleRow packs: partitions = [ci(64) x ky-half], j = second tap dim
    # MM1: j -> kx in {0,1} over ky-halves {0,1}
    w2d1 = np.zeros((128, 2, 128), f32)
    for h in range(2):
        for j in range(2):
            w2d1[64 * h:64 * (h + 1), j] = sw2[:, :, h, j].T
    # MM2: j0 -> (ky=h, kx=2); j1 -> (2,0) on lo half, zero on hi half
    w2d2 = np.zeros((128, 2, 128), f32)
    for h in range(2):
        w2d2[64 * h:64 * (h + 1), 0] = sw2[:, :, h, 2].T
    w2d2[0:64, 1] = sw2[:, :, 2, 0].T
    # MM3 (lo partitions only): j0 -> (2,1); j1 -> (2,2)
    w2d3 = np.zeros((64, 2, 128), f32)
    w2d3[:, 0] = sw2[:, :, 2, 1].T
    w2d3[:, 1] = sw2[:, :, 2, 2].T

    # L3 DR packs: w3d[kx]: j=ky in {0,1}; w3d3: j=kx in {0,1} at ky=2
    w3d = np.zeros((128, 3, 2, 128), f32)
    for kx in range(3):
        for j in range(2):
            w3d[:, kx, j] = sw3[:, :, j, kx].T
    w3d3 = np.zeros((128, 2, 128), f32)
    for j in range(2):
        w3d3[:, j] = sw3[:, :, 2, j].T
    w3s = np.ascontiguousarray(sw3[:, :, 2, 2].T)

    # L4 DR packs: w4d[kx]: j=ky in {0,1}; w4d3: j=kx in {0,1} at ky=2
    w4da = np.zeros((128, 3, 2, 128), f32)
    w4db = np.zeros((128, 3, 2, 64), f32)
    for kx in range(3):
        for j in range(2):
            w4da[:, kx, j] = sw4[:128, :, j, kx].T
            w4db[:, kx, j] = sw4[128:, :, j, kx].T
    w4d3a = np.zeros((128, 2, 128), f32)
    w4d3b = np.zeros((128, 2, 64), f32)
    for j in range(2):
        w4d3a[:, j] = sw4[:128, :, 2, j].T
        w4d3b[:, j] = sw4[128:, :, 2, j].T
    w4sa = np.ascontiguousarray(sw4[:128, :, 2, 2].T)   # [128, 128]
    w4sb = np.ascontiguousarray(sw4[128:, :, 2, 2].T)   # [128, 64]

    # L5 DR packs: j0 = channels 0-127; j1 = channels 128-191 (partitions
    # 0-63, zeros above)
    w5 = sw5[:, :, 0, 0]                               # [co=192, ci=192]
    w5da = np.zeros((128, 2, 128), f32)
    w5da[:, 0] = w5[:128, :128].T
    w5da[0:64, 1] = w5[:128, 128:].T
    w5db = np.zeros((128, 2, 64), f32)
    w5db[:, 0] = w5[128:, :128].T
    w5db[0:64, 1] = w5[128:, 128:].T

    fc_w = np.asarray(inputs["fc_w"], f32)
    c6w = np.asarray(inputs["conv6_w"], f32)[:, :, 0, 0]   # [12, 192]
    Wp = (fc_w @ c6w) / np.float32(NPOS)               # [12, 192]
    cvec = fc_w @ np.asarray(inputs["conv6_b"], f32) + np.asarray(
        inputs["fc_b"], f32)                           # [12]
    wta = np.ascontiguousarray(Wp[:, :128].T)          # [128, 12]
    wtb = np.zeros((65, 12), f32)
    wtb[:64] = Wp[:, 128:].T
    wtb[64] = cvec

    shared = {
        "w1t4": w1t4.astype(f32),
        "w2d1": w2d1.reshape(128, 256).astype(fp8),
        "w2d2": w2d2.reshape(128, 256).astype(fp8),
        "w2d3": w2d3.reshape(64, 256).astype(fp8),
        "w3d": w3d.reshape(128, 768).astype(fp8),
        "w3d3": w3d3.reshape(128, 256).astype(fp8),
        "w3s": w3s.astype(fp8),
        "w4da": w4da.reshape(128, 768).astype(fp8),
        "w4d3a": w4d3a.reshape(128, 256).astype(fp8),
        "w4sa": w4sa.astype(fp8),
        "w4db": w4db.reshape(128, 384).astype(fp8),
        "w4d3b": w4d3b.reshape(128, 128).astype(fp8),
        "w4sb": w4sb.astype(fp8),
        "w5da": w5da.reshape(128, 256).astype(fp8),
        "w5db": w5db.reshape(128, 128).astype(fp8),
        "t1": t1.astype(f32), "t2": t2.astype(f32), "t3": t3.astype(f32),
        "t4a": t4[:128].reshape(128, 1).astype(f32),
        "t4b": t4[128:].reshape(64, 1).astype(f32),
        "a5a": a5[:128].reshape(128, 1).astype(f32),
        "a5b": a5[128:].reshape(64, 1).astype(f32),
        "b5a": b5[:128].reshape(128, 1).astype(f32),
        "b5b": b5[128:].reshape(64, 1).astype(f32),
        "wta": wta.astype(f32), "wtb": wtb.astype(f32),
    }
    # host im2col: cols[b, 3*ky+kx, y*64+x] = xpad[b, 2y+ky, 2x+kx]
    xpad = np.pad(x[:, 0], ((0, 0), (1, 1), (1, 1)))
    cols = np.stack([xpad[:, ky:ky + 127:2, kx:kx + 127:2]
                     for ky in range(3) for kx in range(3)],
                    axis=1).reshape(x.shape[0], 9, 4096)
    in_maps = []
    for c in range(N_CORES):
        m = dict(shared)
        m["x"] = np.ascontiguousarray(cols[c * B:(c + 1) * B])
        in_maps.append(m)
    return in_maps


def kernel(**inputs):
    cache = _build()
    in_maps = _host_prep(inputs)
    res = run_bass_kernel_spmd(cache["nc1"], in_maps,
                               core_ids=list(range(N_CORES)))
    _CACHE["last_results"] = res
    return np.concatenate([res.results[c]["y"] for c in range(N_CORES)],
                          axis=0)


# ---------------------------------------------------------------------------
# numpy golden model of the device algorithm (for fast validation in test.py)
# ---------------------------------------------------------------------------
def golden(inputs):
    f32 = np.float32
    in_maps = _host_prep(inputs)
    outs = []
    for m in in_maps:
        cols = np.asarray(m["x"], f32)  # [B, 9, 4096] host im2col
        t1 = m["t1"][:, 0]
        w1t = m["w1t4"][0:9]
        c1 = np.einsum("btn,tc->bcn", cols, w1t).reshape(-1, 64, 64, 64)
        a1 = (c1 > t1[None, :, None, None]).astype(f32)

        def bconv(a, wt, stride):
            # a: [B,C,H,W] binary; wt[ci, t, co] for t = 3*ky+kx
            Bn, C, H, W = a.shape
            ap = np.pad(a, ((0, 0), (0, 0), (1, 1), (1, 1)))
            Ho, Wo = H // stride, W // stride
            out = np.zeros((Bn, wt.shape[2], Ho, Wo), f32)
            for t in range(9):
                ky, kx = t // 3, t % 3
                sl = ap[:, :, ky:ky + H:stride, kx:kx + W:stride][
                    :, :, :Ho, :Wo]
                out += np.einsum("bcyx,cd->bdyx", sl, wt[:, t])
            return out

        # reconstruct w2 [ci, t, co] from the DR packs
        w2d1 = np.asarray(m["w2d1"], f32).reshape(128, 2, 128)
        w2d2 = np.asarray(m["w2d2"], f32).reshape(128, 2, 128)
        w2d3 = np.asarray(m["w2d3"], f32).reshape(64, 2, 128)
        w2 = np.zeros((64, 9, 128), f32)
        for h in range(2):
            for j in range(2):
                w2[:, 3 * h + j] = w2d1[64 * h:64 * h + 64, j]
            w2[:, 3 * h + 2] = w2d2[64 * h:64 * h + 64, 0]
        w2[:, 6] = w2d2[0:64, 1]
        w2[:, 7] = w2d3[:, 0]
        w2[:, 8] = w2d3[:, 1]
        c2 = bconv(a1, w2, 1)
        a2 = (c2 > m["t2"].reshape(1, 128, 1, 1)).astype(f32)

        w3dg = np.asarray(m["w3d"], f32).reshape(128, 3, 2, 128)
        w3d3g = np.asarray(m["w3d3"], f32).reshape(128, 2, 128)
        w3 = np.zeros((128, 9, 128), f32)
        for kx in range(3):
            for j in range(2):
                w3[:, 3 * j + kx] = w3dg[:, kx, j]
        w3[:, 6] = w3d3g[:, 0]
        w3[:, 7] = w3d3g[:, 1]
        w3[:, 8] = np.asarray(m["w3s"], f32)
        c3 = bconv(a2, w3, 2)
        a3 = (c3 > m["t3"].reshape(1, 128, 1, 1)).astype(f32)

        # reconstruct w4 [ci, t, co] from DR packs + solo
        w4da = np.asarray(m["w4da"], f32).reshape(128, 3, 2, 128)
        w4db = np.asarray(m["w4db"], f32).reshape(128, 3, 2, 64)
        w4d3a = np.asarray(m["w4d3a"], f32).reshape(128, 2, 128)
        w4d3b = np.asarray(m["w4d3b"], f32).reshape(128, 2, 64)
        w4 = np.zeros((128, 9, 192), f32)
        for kx in range(3):
            for j in range(2):
                w4[:, 3 * j + kx, :128] = w4da[:, kx, j]
                w4[:, 3 * j + kx, 128:] = w4db[:, kx, j]
        for j in range(2):
            w4[:, 6 + j, :128] = w4d3a[:, j]
            w4[:, 6 + j, 128:] = w4d3b[:, j]
        w4[:, 8, :128] = np.asarray(m["w4sa"], f32)
        w4[:, 8, 128:] = np.asarray(m["w4sb"], f32)
        c4 = bconv(a3, w4, 1)
        a4 = np.concatenate([
            (c4[:, :128] > m["t4a"].reshape(1, 128, 1, 1)).astype(f32),
            (c4[:, 128:] > m["t4b"].reshape(1, 64, 1, 1)).astype(f32)], axis=1)

        w5dag = np.asarray(m["w5da"], f32).reshape(128, 2, 128)
        w5dbg = np.asarray(m["w5db"], f32).reshape(128, 2, 64)
        w5 = np.zeros((192, 192), f32)  # [ci, co]
        w5[:128, :128] = w5dag[:, 0]
        w5[128:, :128] = w5dag[0:64, 1]
        w5[:128, 128:] = w5dbg[:, 0]
        w5[128:, 128:] = w5dbg[0:64, 1]
        c5 = np.einsum("bcyx,cd->bdyx", a4, w5)
        a5v = np.concatenate([m["a5a"], m["a5b"]], axis=0).reshape(1, 192, 1, 1)
        b5v = np.concatenate([m["b5a"], m["b5b"]], axis=0).reshape(1, 192, 1, 1)
        h5 = np.maximum(a5v * c5 + b5v, 0.0)
        sums = h5.sum(axis=(2, 3))  # [B, 192]
        WT = np.concatenate([m["wta"], m["wtb"][:64]], axis=0)  # [192, 12]
        logits = sums @ WT + m["wtb"][64][None, :]
        z = logits - logits.max(axis=1, keepdims=True)
        e = np.exp(z)
        outs.append(e / e.sum(axis=1, keepdims=True))
    return np.concatenate(outs, axis=0)
